# revision 1
# baseline (speedup 1.0000x reference)
"""Distributed 2-layer GCN (BangaloreGCN) on 8 Trainium2 NeuronCores.

Source-partitioned design (v2):
  * Nodes are assigned contiguously to cores (6250 real + 22 spare slots
    per core, 6272 slots/core, 50176 global slots).  Each core owns the
    edges whose SOURCE node it owns, so every gather is from a small
    LOCAL table ([6272, 256B] rows in core-local DRAM) -- no AllGather.
  * Per layer: dense part (h = x @ W, BN folded) computed per-core for
    own nodes only; message table = dinv * h written to local DRAM;
    per-edge flow is dma_gather (table rows, edge order) followed by
    dma_scatter_add into a global-slot accumulator in local DRAM.
    A ReduceScatter (bf16, add) combines the 8 partial accumulators and
    hands each core exactly its own slots.  Collective payload per core:
    6.4 MB in / 0.8 MB out (vs 12.8 MB AllGather in the v1 design).
  * dma_scatter_add loses updates when one call repeats a destination
    (RMW race on HW), so edges are partitioned into rounds: round r
    holds each destination's r-th incoming edge (per core).  Calls are
    (round, parity, chunk) segments; destination parity selects a
    +-128B base offset so a contiguous [50176, 64] bf16 accumulator can
    be indexed with the required 256B stride.  Shortfalls against the
    global per-call maximum are padded with dummy edges (zero source
    row, unique unused destination).
  * Layer 2 reuses the identical edge order, gather indices and scatter
    indices; only elem_size differs (32 features).
"""

import sys

sys.path.insert(0, "/opt/trn_rl_repo")

import ml_dtypes
import numpy as np

F16 = np.float16

# ---- problem constants ----
N_NODES = 50000
IN_CH = 128
HID = 64
HID2 = 32
BN_EPS = 1e-5

NCORES = 8
P = 128
TILES = 49
SPC = TILES * P            # 6272 slots per core
NSLOT = NCORES * SPC       # 50176
REAL = N_NODES // NCORES   # 6250 real nodes per core
TBW = 128                  # table row width in bf16 elems (256B stride)
CHUNK = int(__import__("os").environ.get("KCHUNK", "12288"))
GCHUNK = int(__import__("os").environ.get("KGCHUNK", "12288"))
DEBUG_DUMP = False


# ----------------------------------------------------------------------
# host-side preparation
# ----------------------------------------------------------------------
def _wrap_idx(arr):
    """[n] int -> [128, n/16] int16 image (16-partition wrap, replicated)."""
    ni = arr.shape[0]
    assert ni % 16 == 0
    blk = arr.reshape(ni // 16, 16).T.astype(np.int16)
    return np.tile(blk, (8, 1))


def _gslot(n):
    return (n // REAL) * SPC + (n % REAL)


def host_prep(x, edge_index, W1, b1, W2, b2, fcW, fcb,
              g1, be1, rm1, rv1, g2, be2, rm2, rv2):
    row = np.asarray(edge_index[0], np.int64)
    col = np.asarray(edge_index[1], np.int64)
    x = np.asarray(x, np.float32)

    deg = np.bincount(col, minlength=N_NODES).astype(np.float32) + 1.0
    dinv = (1.0 / np.sqrt(deg)).astype(np.float32)

    owner = row // REAL
    src_loc = (row % REAL).astype(np.int64)          # local table row
    dst_g = _gslot(col)                               # global dest slot

    # ---- rounds: rank of each edge among its (owner, dest) group ----
    order = np.lexsort((dst_g, owner))
    ow_s, sl_s, dg_s = owner[order], src_loc[order], dst_g[order]
    grp = np.empty(len(order), bool)
    grp[0] = True
    grp[1:] = (ow_s[1:] != ow_s[:-1]) | (dg_s[1:] != dg_s[:-1])
    gid = np.cumsum(grp) - 1
    first = np.flatnonzero(grp)
    rank = np.arange(len(order)) - first[gid]         # occurrence index
    par = (dg_s & 1).astype(np.int64)

    # per (core, rank, parity) counts -> static call sizes
    NR = int(rank.max()) + 1
    key = (ow_s * NR + rank) * 2 + par
    counts = np.bincount(key, minlength=NCORES * NR * 2).reshape(NCORES, NR, 2)
    callmax = counts.max(axis=0)                      # [NR, 2]
    callpad = ((callmax + 127) // 128 * 128).astype(np.int64)

    # split oversized segments into chunks of <= CHUNK
    call_list = []                                    # (rank, parity, size)
    for r in range(NR):
        for p in (0, 1):
            n = int(callpad[r, p])
            if n == 0:
                continue
            while n > 0:
                c = min(n, CHUNK)
                call_list.append((r, p, c))
                n -= c
    tot_idx = sum(c for _, _, c in call_list)

    # group consecutive scatter sub-calls into larger gather bins
    gbins = []                                        # (pos, size, [(p, rel, sz)])
    cur, cur_pos, cur_sz = [], 0, 0
    pos = 0
    for (r, p, size) in call_list:
        if cur_sz + size > GCHUNK:
            gbins.append((cur_pos, cur_sz, cur))
            cur, cur_pos, cur_sz = [], pos, 0
        cur.append((p, cur_sz, size))
        cur_sz += size
        pos += size
    if cur:
        gbins.append((cur_pos, cur_sz, cur))

    # ---- per-core gather/scatter index streams ----
    seg_sorted = np.lexsort((dg_s, par, rank, ow_s))  # core, rank, parity, dest
    ow2 = ow_s[seg_sorted]
    rk2 = rank[seg_sorted]
    pr2 = par[seg_sorted]
    sl2 = sl_s[seg_sorted]
    dg2 = dg_s[seg_sorted]

    gidx = np.zeros((NCORES, tot_idx), np.int64)
    sidx = np.zeros((NCORES, tot_idx), np.int64)
    spare_src = REAL                                   # zero table row
    core_starts = np.searchsorted(ow2, np.arange(NCORES + 1))
    for c in range(NCORES):
        lo, hi = core_starts[c], core_starts[c + 1]
        rk_c, pr_c, sl_c, dg_c = rk2[lo:hi], pr2[lo:hi], sl2[lo:hi], dg2[lo:hi]
        # segment boundaries per (rank, parity)
        seg_key = rk_c * 2 + pr_c
        pos = 0
        consumed = {}
        for r, p, size in call_list:
            # note: consecutive calls with same (r,p) are chunk splits
            k = r * 2 + p
            seg_lo = int(np.searchsorted(seg_key, k, side="left"))
            seg_hi = int(np.searchsorted(seg_key, k, side="right"))
            done = consumed.get(k, 0)
            take = min(size, seg_hi - seg_lo - done)
            a = seg_lo + done
            gidx[c, pos:pos + take] = sl_c[a:a + take]
            sidx[c, pos:pos + take] = dg_c[a:a + take]
            need = size - take
            if need > 0:
                # dummy edges: zero source row, unique unused dest of parity p
                used = np.zeros(NSLOT // 2 + 1, bool)
                used[dg_c[a:a + take] >> 1] = True
                free = np.flatnonzero(~used[:NSLOT // 2])
                dsel = free[:need] * 2 + p
                gidx[c, pos + take:pos + size] = spare_src
                sidx[c, pos + take:pos + size] = dsel
            consumed[k] = done + take
            pos += size
        assert pos == tot_idx

    # ---- BN folding ----
    S1c = (np.asarray(g1) / np.sqrt(np.asarray(rv1) + BN_EPS)).astype(np.float32)
    T1 = ((np.asarray(b1) - np.asarray(rm1)) * S1c + np.asarray(be1)).astype(np.float32)
    S2c = (np.asarray(g2) / np.sqrt(np.asarray(rv2) + BN_EPS)).astype(np.float32)
    T2 = ((np.asarray(b2) - np.asarray(rm2)) * S2c + np.asarray(be2)).astype(np.float32)
    W1p = (np.asarray(W1) * S1c[None, :]).astype(np.float32)
    W2p = (np.asarray(W2) * S2c[None, :]).astype(np.float32)

    # ---- per-core tensors ----
    in_maps = []
    for c in range(NCORES):
        nodes = np.arange(c * REAL, (c + 1) * REAL)
        xs = np.zeros((SPC, IN_CH), np.float32)
        xs[:REAL] = x[nodes] * dinv[nodes, None]
        dv = np.zeros(SPC, np.float32)
        dv[:REAL] = dinv[nodes]
        in_maps.append({
            "xT": np.ascontiguousarray(xs.T).astype(F16),
            "gidx": _wrap_idx(gidx[c]),
            "sidx": _wrap_idx(sidx[c] >> 1),
            "soff": (sidx[c] & 1),          # parity per edge (host-only check)
            "dinv": np.ascontiguousarray(dv.reshape(TILES, P).T).astype(F16),
            "w1": W1p.astype(F16),
            "w2": W2p.astype(F16),
            "t1": np.tile(T1[None, :], (P, 1)).astype(F16),
            "t2": np.tile(T2[None, :], (P, 1)).astype(F16),
            "fcw": np.tile(np.asarray(fcW, np.float32).reshape(1, -1), (P, 1)).astype(F16),
        })
    # parities are per-call constants; verify and strip
    pos = 0
    call_par = []
    for r, p, size in call_list:
        for c in range(NCORES):
            assert (in_maps[c]["soff"][pos:pos + size] == p).all()
        call_par.append(p)
        pos += size
    for m in in_maps:
        del m["soff"]

    consts = dict(call_list=call_list, gbins=gbins, tot_idx=tot_idx,
                  fcb=float(np.asarray(fcb).reshape(-1)[0]))
    return in_maps, consts


# ----------------------------------------------------------------------
# raw dma_gather (elem_size below 256B; stride multiple of 256B)
# ----------------------------------------------------------------------
def _dma_gather_raw(gp, bassmod, out_ap, in_ap, idxs_ap, num_idxs, elem_size,
                    elem_step, single_packet=True, queue_num=0):
    import concourse.mybir as mybir
    from concourse import ap_utils
    from concourse.bass import MemorySpace, exact_div, round_up_to_multiple

    assert idxs_ap.dtype == mybir.dt.int16
    assert in_ap.dtype == out_ap.dtype
    assert in_ap.space == MemorySpace.DRAM
    assert idxs_ap.space == MemorySpace.SBUF and out_ap.space == MemorySpace.SBUF
    assert ap_utils.ap_is_contiguous(out_ap.ap[1:])
    assert ap_utils.ap_is_contiguous(idxs_ap.ap[1:])
    assert in_ap.ap[-1][1] == out_ap.ap[-1][1] == elem_size
    assert out_ap.ap[0][1] * out_ap.ap[1][1] == round_up_to_multiple(num_idxs, 128)
    assert in_ap.ap[0][0] == elem_step
    stride_bytes_256 = exact_div(elem_step * mybir.dt.size(in_ap.dtype), 256)
    assert stride_bytes_256 < 256
    return gp.add_instruction(
        mybir.InstDMAGatherAnt(
            name=bassmod.get_next_instruction_name(),
            ins=[*gp.lower_ap_dma(in_ap, for_custom_bir_dma=True),
                 gp.lower_ap(idxs_ap),
                 gp.lower_val_access(gp.to_reg(num_idxs))],
            outs=[gp.lower_ap(out_ap)],
            transpose=False,
            num_idxs=num_idxs,
            elem_size=elem_size,
            stride_bytes_256=stride_bytes_256,
            gen_mode=0,
            single_packet=single_packet,
            queue_num=queue_num,
            sbuf_tokens_per_rank=0,
            sbuf_free_dim_per_rank=0,
            sbuf_free_dim_pad_per_rank=0,
            sbuf_byte_offset=0,
        ))


# ----------------------------------------------------------------------
# device program
# ----------------------------------------------------------------------
def build_bass(call_list, gbins, tot_idx):
    import concourse.bacc as bacc
    import concourse.bass as bassm
    import concourse.mybir as mybir
    import concourse.tile as tile
    from concourse.library_config import mlp
    from concourse.masks import make_identity

    f32 = mybir.dt.float32
    bf = mybir.dt.float16
    i16 = mybir.dt.int16

    import os as _os
    nc = bacc.Bacc("TRN2", target_bir_lowering=False,
                   dynamic_dma_scratch_size=int(_os.environ.get("KSCRATCH", "49152")),
                   num_swdge_queues=int(_os.environ.get("KNQ", "2")))
    xT_d = nc.dram_tensor("xT", [P, SPC], bf, kind="ExternalInput")
    gidx_d = nc.dram_tensor("gidx", [P, tot_idx // 16], i16, kind="ExternalInput")
    sidx_d = nc.dram_tensor("sidx", [P, tot_idx // 16], i16, kind="ExternalInput")
    dinv_d = nc.dram_tensor("dinv", [P, TILES], bf, kind="ExternalInput")
    w1_d = nc.dram_tensor("w1", [IN_CH, HID], bf, kind="ExternalInput")
    w2_d = nc.dram_tensor("w2", [HID, HID2], bf, kind="ExternalInput")
    t1_d = nc.dram_tensor("t1", [P, HID], bf, kind="ExternalInput")
    t2_d = nc.dram_tensor("t2", [P, HID2], bf, kind="ExternalInput")
    fcw_d = nc.dram_tensor("fcw", [P, HID2], bf, kind="ExternalInput")
    y_d = nc.dram_tensor("y", [P, TILES], f32, kind="ExternalOutput")

    with tile.TileContext(nc) as tc:
        with (
            tc.tile_pool(name="const", bufs=1) as cpool,
            tc.tile_pool(name="work", bufs=1) as upool,
            tc.tile_pool(name="g", bufs=int(__import__("os").environ.get("KGBUF", "4"))) as gpool,
            tc.tile_pool(name="tmp", bufs=1) as wpool,
            tc.tile_pool(name="pmm", bufs=2, space="PSUM") as pmm,
            tc.tile_pool(name="ptr", bufs=2, space="PSUM") as ptr,
            tc.tile_pool(name="dram", bufs=1, space="DRAM") as dpool,
        ):
            nc.gpsimd.load_library(mlp)

            # ---- DRAM scratch ----
            tab1_d = dpool.tile([SPC, TBW], bf)
            tab2_d = dpool.tile([SPC, TBW], bf)
            acc1_d = dpool.tile([NSLOT // 2, 2 * HID], bf)    # [50176,64] packed
            acc2_d = dpool.tile([NSLOT // 2, 2 * HID], bf)    # layer2: 32 used of each 64
            rs1_d = dpool.tile([NSLOT // 2 // NCORES, 2 * HID], bf)
            rs2_d = dpool.tile([NSLOT // 2 // NCORES, 2 * HID], bf)

            # ---- constants ----
            gidx_t = cpool.tile([P, tot_idx // 16], i16)
            nc.scalar.dma_start(out=gidx_t[:], in_=gidx_d[:])
            sidx_t = cpool.tile([P, tot_idx // 16], i16)
            nc.scalar.dma_start(out=sidx_t[:], in_=sidx_d[:])
            dinv_t = cpool.tile([P, TILES], bf)
            nc.scalar.dma_start(out=dinv_t[:], in_=dinv_d[:])
            w1_t = cpool.tile([IN_CH, HID], bf)
            nc.scalar.dma_start(out=w1_t[:], in_=w1_d[:])
            w2_t = cpool.tile([HID, HID2], bf)
            nc.scalar.dma_start(out=w2_t[:], in_=w2_d[:])
            t1_t = cpool.tile([P, HID], bf)
            nc.scalar.dma_start(out=t1_t[:], in_=t1_d[:])
            t2_t = cpool.tile([P, HID2], bf)
            nc.scalar.dma_start(out=t2_t[:], in_=t2_d[:])
            fcw_t = cpool.tile([P, HID2], bf)
            nc.scalar.dma_start(out=fcw_t[:], in_=fcw_d[:])
            ident = cpool.tile([P, P], bf)
            make_identity(nc, ident[:])

            # dinv squared image
            dinv2_t = cpool.tile([P, TILES], bf)
            nc.vector.tensor_tensor(out=dinv2_t[:], in0=dinv_t[:], in1=dinv_t[:],
                                    op=mybir.AluOpType.mult)

            # ---- zero acc1: contiguous 1KB elements (no small-xfer penalty)
            ZW = 512
            ZROWS = NSLOT // 2 // P * 2 * HID // ZW      # blocks per partition
            zt = cpool.tile([P, ZW], bf)
            nc.gpsimd.memset(zt[:], 0.0)

            # ---- L1 dense: u1 = (dinv*x)^T tiles @ W1p ----
            xfull = cpool.tile([P, SPC], bf)
            for lo, w in ((0, SPC // 2), (SPC // 2, SPC - SPC // 2)):
                nc.sync.dma_start(out=xfull[:, lo:lo + w], in_=xT_d[:, lo:lo + w])
            u1_t = upool.tile([P, TILES * HID], bf, tag="u1")
            for b in range((TILES + 7) // 8):
                pm = pmm.tile([P, 512], f32, space="PSUM", tag="pm")
                ts = range(b * 8, min((b + 1) * 8, TILES))
                for i, t in enumerate(ts):
                    nc.tensor.matmul(out=pm[:, i * HID:(i + 1) * HID],
                                     lhsT=xfull[:, t * P:(t + 1) * P],
                                     rhs=w1_t[:], start=True, stop=True)
                nts = len(ts)
                nc.vector.tensor_copy(
                    out=u1_t[:, b * 8 * HID:(b * 8 + nts) * HID],
                    in_=pm[:, 0:nts * HID])

            # zero acc1 now (dep on u1 junk): the input loads, dense pass and
            # table write get the DMA device first; this transfer overlaps the
            # first gather's desc-gen.
            zt1 = cpool.tile([P, ZW], bf)
            nc.vector.tensor_tensor(
                out=zt1[:], in0=zt[:],
                in1=u1_t[:, 0:1].to_broadcast([P, ZW]),
                op=mybir.AluOpType.mult)
            nc.scalar.dma_start(
                out=bassm.AP(tensor=acc1_d[:].tensor, offset=0,
                             ap=[[ZROWS * ZW, P], [ZW, ZROWS], [1, ZW]]),
                in_=zt1[:].rearrange("p (a w) -> p a w", a=1)
                    .to_broadcast([P, ZROWS, ZW]),
            )

            # table1 = u1 rows (strided 256B)
            nc.sync.dma_start(
                out=tab1_d[:].rearrange("(t p) w -> p t w", p=P)[:, :, 0:HID],
                in_=u1_t[:].rearrange("p (t f) -> p t f", f=HID),
            )

            # ---- per-edge streams ----
            def edge_stream(tab, acc, fw, layer):
                """gather fw-wide rows from tab (large bins) and scatter-add
                into acc (<=4096-idx sub-calls, unique dests per call).

                Scatters use prepare_only + per-call trigger_dma so the Q7
                descriptor generation runs while the previous scatter's
                transfer is still in flight; an explicit dma_sem chain keeps
                the transfers ordered (cross-call RMW hazard).  The next
                bin's gather is issued before the previous bin's scatters."""
                pending = None

                def flush(gv, subs, bpos):
                    for (p, rel, sz) in subs:
                        acc_ap = bassm.AP(tensor=acc[:].tensor, offset=p * HID,
                                          ap=[[2 * HID, NSLOT // 2], [1, fw]])
                        nc.gpsimd.dma_scatter_add(
                            acc_ap, gv[:, rel // P:(rel + sz) // P, :],
                            sidx_t[:, (bpos + rel) // 16:(bpos + rel + sz) // 16],
                            sz, sz, fw, elem_step=2 * HID, single_packet=True,
                            queue_num=int(_os.environ.get("KNQ", "1")) - 1)

                for (bpos, bsz, subs) in gbins:
                    g_t = gpool.tile([P, GCHUNK // P, fw], bf, tag="g")
                    gv = g_t[:, 0:bsz // P, :]
                    _dma_gather_raw(
                        nc.gpsimd, nc, gv,
                        bassm.AP(tensor=tab[:].tensor, offset=0,
                                 ap=[[TBW, SPC], [1, fw]]),
                        gidx_t[:, bpos // 16:(bpos + bsz) // 16], bsz, fw, TBW,
                        single_packet=False, queue_num=0)
                    if pending is not None:
                        flush(*pending)
                    pending = (gv, subs, bpos)
                if pending is not None:
                    flush(*pending)

            edge_stream(tab1_d, acc1_d, HID, 1)

            # ---- RS1 + post1: z = relu(dinv*agg + dinv2*u1 + T1) ----
            nc.gpsimd.collective_compute(
                "ReduceScatter", mybir.AluOpType.add,
                replica_groups=[list(range(NCORES))],
                ins=[acc1_d[:]], outs=[rs1_d[:]],
            )
            # zero acc2 inside the RS1 window: zt2 = zt * (junk read from
            # acc1) gives a true data dependency on the last L1 scatter, so
            # Tile cannot hoist this transfer into the L1 stream.
            zt2pre = cpool.tile([P, 4], bf)
            nc.sync.dma_start(
                out=zt2pre[:],
                in_=bassm.AP(tensor=acc1_d[:].tensor, offset=0,
                             ap=[[2 * HID, P], [1, 4]]))
            zt2 = cpool.tile([P, ZW], bf)
            nc.vector.tensor_tensor(
                out=zt2[:], in0=zt[:],
                in1=zt2pre[:, 0:1].to_broadcast([P, ZW]),
                op=mybir.AluOpType.mult)
            nc.scalar.dma_start(
                out=bassm.AP(tensor=acc2_d[:].tensor, offset=0,
                             ap=[[ZROWS * ZW, P], [ZW, ZROWS], [1, ZW]]),
                in_=zt2[:].rearrange("p (a w) -> p a w", a=1)
                    .to_broadcast([P, ZROWS, ZW]),
            )
            agg1 = upool.tile([P, TILES * HID], bf, tag="agg1")
            for lo, nt in ((0, 24), (24, TILES - 24)):
                nc.sync.dma_start(
                    out=agg1[:, lo * HID:(lo + nt) * HID]
                        .rearrange("p (t f) -> p t f", f=HID),
                    in_=bassm.AP(tensor=rs1_d[:].tensor, offset=lo * HID * P,
                                 ap=[[HID, P], [HID * P, nt], [1, HID]]),
                )

            def precompute_self(u, dv2t, tt, fw, tag):
                # self-term + bias: only needs u, so it runs before the
                # ReduceScatter while the engines are otherwise idle
                pre = wpool.tile([P, TILES, fw], bf, tag=f"pre{tag}")
                u3 = u[:].rearrange("p (t f) -> p t f", f=fw)
                nc.vector.tensor_tensor(
                    out=pre[:], in0=u3,
                    in1=dv2t[:, :, None].to_broadcast([P, TILES, fw]),
                    op=mybir.AluOpType.mult)
                nc.vector.tensor_tensor(
                    out=pre[:], in0=pre[:],
                    in1=tt[:, None, :].to_broadcast([P, TILES, fw]),
                    op=mybir.AluOpType.add)
                return pre

            def post(agg, pre, dvt, fw, out_t):
                # two tile-halves so DVE/ACT pipeline (consumers of half 0
                # can start while half 1 is still in the DVE chain)
                a3 = agg[:].rearrange("p (t f) -> p t f", f=fw)
                H0 = 24
                for h, (lo, nt) in enumerate(((0, H0), (H0, TILES - H0))):
                    tmp = wpool.tile([P, nt, fw], bf, tag=f"pa{fw}h{h}")
                    nc.vector.tensor_tensor(
                        out=tmp[:], in0=a3[:, lo:lo + nt, :],
                        in1=dvt[:, lo:lo + nt, None].to_broadcast([P, nt, fw]),
                        op=mybir.AluOpType.mult)
                    nc.vector.tensor_tensor(out=tmp[:], in0=tmp[:],
                                            in1=pre[:, lo:lo + nt, :],
                                            op=mybir.AluOpType.add)
                    nc.scalar.activation(
                        out=out_t[:, lo * fw:(lo + nt) * fw],
                        in_=tmp[:].rearrange("p t f -> p (t f)"),
                        func=mybir.ActivationFunctionType.Relu)

            # u1 is pre-scaled by dinv (host scales x), so its self-term
            # multiplier is dinv, not dinv^2.
            pre1 = precompute_self(u1_t, dinv_t, t1_t, HID, "1")
            z_t = upool.tile([P, TILES * HID], bf, tag="z")
            post(agg1, pre1, dinv_t, HID, z_t)

            # ---- L2 dense: u2 = z @ W2p (via PE transpose) ----
            zT_t = upool.tile([HID, TILES * P], bf, tag="zT")
            for b in range((TILES + 3) // 4):        # 4 tiles per PSUM bank
                tr = ptr.tile([HID, 512], bf, space="PSUM", tag="tr")
                ts = range(b * 4, min((b + 1) * 4, TILES))
                for i, t in enumerate(ts):
                    nc.tensor.transpose(out=tr[:, i * P:(i + 1) * P],
                                        in_=z_t[:, t * HID:(t + 1) * HID],
                                        identity=ident[:])
                nts = len(ts)
                nc.vector.tensor_copy(out=zT_t[:, b * 4 * P:(b * 4 + nts) * P],
                                      in_=tr[:, 0:nts * P])
            u2_t = upool.tile([P, TILES * HID2], bf, tag="u2")
            t2v = wpool.tile([P, TILES, HID2], bf, tag="t2v")
            tab2_v = tab2_d[:].rearrange("(t p) w -> p t w", p=P)[:, :, 0:HID2]
            for b in range((TILES + 15) // 16):
                pm = pmm.tile([P, 512], f32, space="PSUM", tag="pm2")
                ts = range(b * 16, min((b + 1) * 16, TILES))
                for i, t in enumerate(ts):
                    nc.tensor.matmul(out=pm[:, i * HID2:(i + 1) * HID2],
                                     lhsT=zT_t[:, t * P:(t + 1) * P],
                                     rhs=w2_t[:], start=True, stop=True)
                nts = len(ts)
                b0 = b * 16
                nc.vector.tensor_copy(out=u2_t[:, b0 * HID2:(b0 + nts) * HID2],
                                      in_=pm[:, 0:nts * HID2])
                # table2 slice = dinv * u2, straight from PSUM
                nc.vector.tensor_tensor(
                    out=t2v[:, b0:b0 + nts, :],
                    in0=pm[:, 0:nts * HID2].rearrange("p (t f) -> p t f", f=HID2),
                    in1=dinv_t[:, b0:b0 + nts, None].to_broadcast([P, nts, HID2]),
                    op=mybir.AluOpType.mult)
                nc.sync.dma_start(out=tab2_v[:, b0:b0 + nts, :],
                                  in_=t2v[:, b0:b0 + nts, :])

            edge_stream(tab2_d, acc2_d, HID2, 2)

            # ---- RS2 + post2 + fc ----
            import os as _os2
            if _os2.environ.get("KRS2S", "0") == "1":
                nc.gpsimd.collective_compute(
                    "ReduceScatter", mybir.AluOpType.add,
                    replica_groups=[list(range(NCORES))],
                    ins=[bassm.AP(tensor=acc2_d[:].tensor, offset=0,
                                  ap=[[2 * HID, NSLOT // 2], [HID, 2], [1, HID2]])],
                    outs=[bassm.AP(tensor=rs2_d[:].tensor, offset=0,
                                   ap=[[2 * HID, NSLOT // 2 // NCORES], [HID, 2], [1, HID2]])],
                )
            else:
                nc.gpsimd.collective_compute(
                    "ReduceScatter", mybir.AluOpType.add,
                    replica_groups=[list(range(NCORES))],
                    ins=[acc2_d[:]], outs=[rs2_d[:]],
                )
            agg2 = upool.tile([P, TILES * HID2], bf, tag="agg2")
            for lo, nt in ((0, 24), (24, TILES - 24)):
                nc.sync.dma_start(
                    out=agg2[:, lo * HID2:(lo + nt) * HID2]
                        .rearrange("p (t f) -> p t f", f=HID2),
                    in_=bassm.AP(tensor=rs2_d[:].tensor, offset=lo * HID * P,
                                 ap=[[HID, P], [HID * P, nt], [1, HID2]]),
                )
            # table2 values dinv*u2 give self term dinv2*u2 = dinv*(dinv*u2):
            # reuse post() with u=u2, dv2t=dinv2.
            pre2 = precompute_self(u2_t, dinv2_t, t2_t, HID2, "2")
            h2_t = upool.tile([P, TILES * HID2], bf, tag="h2")
            post(agg2, pre2, dinv_t, HID2, h2_t)

            # fc: y = sum_f h2 * fcW
            prod = wpool.tile([P, TILES, HID2], bf, tag="prod")
            nc.vector.tensor_tensor(
                out=prod[:], in0=h2_t[:].rearrange("p (t f) -> p t f", f=HID2),
                in1=fcw_t[:, None, :].to_broadcast([P, TILES, HID2]),
                op=mybir.AluOpType.mult)
            out_t = upool.tile([P, TILES], f32, tag="out")
            nc.vector.reduce_sum(out=out_t[:, :, None], in_=prod[:],
                                 axis=mybir.AxisListType.X)
            nc.sync.dma_start(out=y_d[:], in_=out_t[:])

            if DEBUG_DUMP:
                for nm, tl in (("dbg_z", z_t), ("dbg_agg1", agg1),
                               ("dbg_u2", u2_t), ("dbg_agg2", agg2),
                               ("dbg_h2", h2_t)):
                    dd = nc.dram_tensor(nm, list(tl[:].shape), tl[:].dtype,
                                        kind="ExternalOutput")
                    nc.gpsimd.dma_start(out=dd[:], in_=tl[:])

    nc.compile()
    return nc


# ----------------------------------------------------------------------
# entry points
# ----------------------------------------------------------------------
def prepare(inputs):
    inputs = {k: np.asarray(v) for k, v in inputs.items()}
    in_maps, consts = host_prep(**inputs)
    nc = build_bass(consts["call_list"], consts["gbins"], consts["tot_idx"])
    return nc, in_maps, consts


def execute(nc, in_maps):
    from concourse.bass_utils import run_bass_kernel_spmd
    return run_bass_kernel_spmd(nc, in_maps, core_ids=list(range(NCORES)))


def unshard(res, consts):
    y = np.zeros((N_NODES, 1), np.float32)
    fcb = consts["fcb"]
    for c in range(NCORES):
        v = np.asarray(res.results[c]["y"], np.float32)   # [P, TILES]
        slots = v.T.reshape(-1)                            # slot t*128+p -> v[p,t]
        y[c * REAL:(c + 1) * REAL, 0] = slots[:REAL] + fcb
    return y


def kernel(**inputs):
    nc, in_maps, consts = prepare(inputs)
    res = execute(nc, in_maps)
    return unshard(res, consts)



# revision 12
# speedup vs baseline: 1.2089x; 1.2089x over previous
"""Distributed 2-layer GCN (BangaloreGCN) on 8 Trainium2 NeuronCores.

Matmul-aggregation design (v3):
  * Source-partitioned: core c owns nodes [c*6250, (c+1)*6250) and the
    edges whose SOURCE it owns.  Per layer, each core computes a local
    message table (dinv-scaled dense transform of its own nodes), then
    gathers per-edge messages with dma_gather in DEST-SORTED order.
  * The scatter side is done on the PE array instead of dma_scatter_add:
    the global dest space is split into 400 windows of 128 slots.  Each
    128-edge gathered tile (edge i -> partition i%128) is multiplied by
    a one-hot "selection" matrix S [128 edges, 128 dests] built on the
    DVE (is_equal of per-edge dest-column vs an iota row), accumulating
    partial sums for a window directly in PSUM.  This removes the
    scatter DMA, the accumulator zeroing, and the scatter descriptor
    generation of v2 entirely.
  * Node -> slot assignment is chosen by a greedy packer so that every
    window needs at most 2 tiles per source core (max in-window edge
    count <= 256 for all 8 cores): the SPMD-static stream is ~103k
    indices per core vs 800k/8 = 100k real edges.
  * Slot labeling ell = p*50 + w makes the per-core accumulator chunk
    contiguous per partition, so the PSUM->DRAM staging writes run at
    full DMA rate, and the ReduceScatter chunk c is exactly core c's
    own slots.  Both layers share the identical edge stream, gather
    indices, and S structure (S is rebuilt per layer; it does not fit
    in SBUF).
"""

import sys

sys.path.insert(0, "/opt/trn_rl_repo")

import ml_dtypes
import numpy as np

F16 = np.float16

# ---- problem constants ----
N_NODES = 50000
IN_CH = 128
HID = 64
HID2 = 32
BN_EPS = 1e-5

NCORES = 8
P = 128
WPC = 50                   # windows (tiles) per core
SPC = P * WPC              # 6400 slots per core
NSLOT = NCORES * SPC       # 51200
NWIN = NCORES * WPC        # 400 global windows
REAL = N_NODES // NCORES   # 6250 real nodes per core
WCAP = 127                 # real nodes per window (p=127 spare everywhere)
TBW = 128                  # table row width in bf16 elems (256B stride)
SPARE_ROW = WCAP * WPC     # a slot that is spare on every core (p=127,w=0)
NTBIN = int(__import__("os").environ.get("KNTBIN", "48"))  # tiles per gather bin


# ----------------------------------------------------------------------
# host-side preparation
# ----------------------------------------------------------------------
def _wrap_idx(arr):
    """[n] int -> [128, n/16] int16 image (16-partition wrap, replicated)."""
    ni = arr.shape[0]
    assert ni % 16 == 0
    blk = arr.reshape(ni // 16, 16).T.astype(np.int16)
    return np.tile(blk, (8, 1))


def _pack_windows(Mi):
    """Greedy: assign nodes (rows of Mi [REAL, 8] = per-source-core indeg)
    to WPC windows, minimizing the max per-core in-window load, capped at
    WCAP nodes per window.  Returns win[i] for nodes in degree-sorted
    order and that order."""
    srt = np.argsort(-Mi.sum(1), kind="stable")
    Ms = Mi[srt]
    loads = np.zeros((WPC, NCORES), np.int64)
    cnt = np.zeros(WPC, np.int64)
    win = np.empty(REAL, np.int64)
    big = 1 << 40
    for i in range(REAL):
        cand = (loads + Ms[i]).max(1) + (cnt >= WCAP) * big
        w = int(np.argmin(cand))
        win[i] = w
        loads[w] += Ms[i]
        cnt[w] += 1
    return srt, win


def host_prep(x, edge_index, W1, b1, W2, b2, fcW, fcb,
              g1, be1, rm1, rv1, g2, be2, rm2, rv2):
    row = np.asarray(edge_index[0], np.int64)
    col = np.asarray(edge_index[1], np.int64)
    x = np.asarray(x, np.float32)

    deg = np.bincount(col, minlength=N_NODES).astype(np.float32) + 1.0
    dinv = (1.0 / np.sqrt(deg)).astype(np.float32)

    owner_src = row // REAL

    # ---- node -> slot assignment (window packing per dest core) ----
    M = np.zeros((N_NODES, NCORES), np.int32)
    np.add.at(M, (col, owner_src), 1)
    slot_of_node = np.full(N_NODES, -1, np.int64)      # global slot
    node_of_slot = np.full((NCORES, SPC), -1, np.int64)
    for c in range(NCORES):
        nodes = np.arange(c * REAL, (c + 1) * REAL)
        srt, win = _pack_windows(M[nodes])
        # p = rank within window (stable in assignment order)
        o2 = np.argsort(win, kind="stable")
        wsort = win[o2]
        first = np.zeros(REAL, np.int64)
        starts = np.r_[0, np.flatnonzero(np.diff(wsort)) + 1]
        first[starts] = starts
        first = np.maximum.accumulate(first)
        p_of = np.empty(REAL, np.int64)
        p_of[o2] = np.arange(REAL) - first
        assert p_of.max() < WCAP
        ell = p_of * WPC + win
        slot_of_node[nodes[srt]] = c * SPC + ell
        node_of_slot[c, ell] = nodes[srt]

    # ---- per-edge window/column/source-row ----
    gdst = slot_of_node[col]
    c_d, ell_d = gdst // SPC, gdst % SPC
    p_d, w_d = ell_d // WPC, ell_d % WPC
    gwin = c_d * WPC + w_d                             # [E] global window
    src_slot = slot_of_node[row] % SPC                 # local table row

    # ---- per-window tile counts (static, max over source cores) ----
    cnts = np.zeros((NWIN, NCORES), np.int64)
    np.add.at(cnts, (gwin, owner_src), 1)
    T = np.maximum(1, -(-cnts.max(1) // P)).astype(np.int64)   # [NWIN]
    win_tile0 = np.r_[0, np.cumsum(T)][:-1]
    ntiles = int(T.sum())
    stream = ntiles * P

    # ---- per-core gather index + dest-column streams ----
    gidx_s = np.full((NCORES, stream), SPARE_ROW, np.int64)
    dloc_s = np.full((NCORES, stream), 255, np.int64)
    for h in range(NCORES):
        sel = owner_src == h
        gw, ss, pd = gwin[sel], src_slot[sel], p_d[sel]
        o = np.argsort(gw, kind="stable")
        gw, ss, pd = gw[o], ss[o], pd[o]
        starts = np.r_[0, np.flatnonzero(np.diff(gw)) + 1]
        first = np.zeros(len(gw), np.int64)
        first[starts] = starts
        first = np.maximum.accumulate(first)
        rank = np.arange(len(gw)) - first
        pos = win_tile0[gw] * P + rank
        gidx_s[h, pos] = ss
        dloc_s[h, pos] = pd

    # ---- gather bins: consecutive whole windows, <= NTBIN tiles ----
    bins = []                                          # (t_lo, nt, w_lo, nw)
    w_lo, t_lo = 0, 0
    for W in range(NWIN):
        if W > w_lo and (win_tile0[W] + T[W] - t_lo) > NTBIN:
            bins.append((t_lo, int(win_tile0[W] - t_lo), w_lo, W - w_lo))
            w_lo, t_lo = W, int(win_tile0[W])
    bins.append((t_lo, ntiles - t_lo, w_lo, NWIN - w_lo))
    assert max(b[1] for b in bins) <= NTBIN

    # ---- BN folding ----
    S1c = (np.asarray(g1) / np.sqrt(np.asarray(rv1) + BN_EPS)).astype(np.float32)
    T1 = ((np.asarray(b1) - np.asarray(rm1)) * S1c + np.asarray(be1)).astype(np.float32)
    S2c = (np.asarray(g2) / np.sqrt(np.asarray(rv2) + BN_EPS)).astype(np.float32)
    T2 = ((np.asarray(b2) - np.asarray(rm2)) * S2c + np.asarray(be2)).astype(np.float32)
    W1p = (np.asarray(W1) * S1c[None, :]).astype(np.float32)
    W2p = (np.asarray(W2) * S2c[None, :]).astype(np.float32)

    # ---- per-core tensors ----
    # xT column j holds slot (j%128)*WPC + j//128 so dense tile t yields
    # u1[p, t*HID:..] = slot p*WPC + t.
    colperm = (np.arange(SPC) % P) * WPC + (np.arange(SPC) // P)
    iota = np.tile(np.arange(P, dtype=np.float32)[None, :], (P, 1))
    in_maps = []
    for c in range(NCORES):
        xs = np.zeros((SPC, IN_CH), np.float32)
        dv = np.zeros(SPC, np.float32)
        valid = node_of_slot[c] >= 0
        nd = node_of_slot[c][valid]
        xs[valid] = x[nd] * dinv[nd, None]
        dv[valid] = dinv[nd]
        xs = xs[colperm]                                # [SPC(col j), IN_CH]
        dv_im = dv.reshape(P, WPC)                      # [p, w]
        in_maps.append({
            "xT": np.ascontiguousarray(xs.T).astype(F16),
            "gidx": _wrap_idx(gidx_s[c]),
            "dloc": np.ascontiguousarray(
                dloc_s[c].reshape(ntiles, P).T).astype(F16),
            "dinv": dv_im.astype(F16),
            "iota": iota.astype(F16),
            "w1": W1p.astype(F16),
            "w2": W2p.astype(F16),
            "t1": np.tile(T1[None, :], (P, 1)).astype(F16),
            "t2": np.tile(T2[None, :], (P, 1)).astype(F16),
            "fcw": np.tile(np.asarray(fcW, np.float32).reshape(1, -1),
                           (P, 1)).astype(F16),
        })

    consts = dict(T=T.tolist(), win_tile0=win_tile0.tolist(), bins=bins,
                  ntiles=ntiles, node_of_slot=node_of_slot,
                  fcb=float(np.asarray(fcb).reshape(-1)[0]))
    return in_maps, consts


# ----------------------------------------------------------------------
# raw dma_gather (elem_size below 256B; stride multiple of 256B)
# ----------------------------------------------------------------------
def _dma_gather_raw(gp, bassmod, out_ap, in_ap, idxs_ap, num_idxs, elem_size,
                    elem_step, single_packet=True, queue_num=0):
    import concourse.mybir as mybir
    from concourse import ap_utils
    from concourse.bass import MemorySpace, exact_div, round_up_to_multiple

    assert idxs_ap.dtype == mybir.dt.int16
    assert in_ap.dtype == out_ap.dtype
    assert in_ap.space == MemorySpace.DRAM
    assert idxs_ap.space == MemorySpace.SBUF and out_ap.space == MemorySpace.SBUF
    assert ap_utils.ap_is_contiguous(out_ap.ap[1:])
    assert ap_utils.ap_is_contiguous(idxs_ap.ap[1:])
    assert in_ap.ap[-1][1] == out_ap.ap[-1][1] == elem_size
    assert out_ap.ap[0][1] * out_ap.ap[1][1] == round_up_to_multiple(num_idxs, 128)
    assert in_ap.ap[0][0] == elem_step
    stride_bytes_256 = exact_div(elem_step * mybir.dt.size(in_ap.dtype), 256)
    assert stride_bytes_256 < 256
    return gp.add_instruction(
        mybir.InstDMAGatherAnt(
            name=bassmod.get_next_instruction_name(),
            ins=[*gp.lower_ap_dma(in_ap, for_custom_bir_dma=True),
                 gp.lower_ap(idxs_ap),
                 gp.lower_val_access(gp.to_reg(num_idxs))],
            outs=[gp.lower_ap(out_ap)],
            transpose=False,
            num_idxs=num_idxs,
            elem_size=elem_size,
            stride_bytes_256=stride_bytes_256,
            gen_mode=0,
            single_packet=single_packet,
            queue_num=queue_num,
            sbuf_tokens_per_rank=0,
            sbuf_free_dim_per_rank=0,
            sbuf_free_dim_pad_per_rank=0,
            sbuf_byte_offset=0,
        ))


# ----------------------------------------------------------------------
# device program
# ----------------------------------------------------------------------
def build_bass(T, win_tile0, bins, ntiles):
    import concourse.bacc as bacc
    import concourse.bass as bassm
    import concourse.mybir as mybir
    import concourse.tile as tile
    from concourse.masks import make_identity

    f32 = mybir.dt.float32
    bf = mybir.dt.float16
    i16 = mybir.dt.int16

    import os as _os
    nc = bacc.Bacc("TRN2", target_bir_lowering=False,
                   dynamic_dma_scratch_size=int(_os.environ.get("KSCRATCH", "49152")),
                   num_swdge_queues=1)
    xT_d = nc.dram_tensor("xT", [P, SPC], bf, kind="ExternalInput")
    gidx_d = nc.dram_tensor("gidx", [P, ntiles * 8], i16, kind="ExternalInput")
    dloc_d = nc.dram_tensor("dloc", [P, ntiles], bf, kind="ExternalInput")
    dinv_d = nc.dram_tensor("dinv", [P, WPC], bf, kind="ExternalInput")
    iota_d = nc.dram_tensor("iota", [P, P], bf, kind="ExternalInput")
    w1_d = nc.dram_tensor("w1", [IN_CH, HID], bf, kind="ExternalInput")
    w2_d = nc.dram_tensor("w2", [HID, HID2], bf, kind="ExternalInput")
    t1_d = nc.dram_tensor("t1", [P, HID], bf, kind="ExternalInput")
    t2_d = nc.dram_tensor("t2", [P, HID2], bf, kind="ExternalInput")
    fcw_d = nc.dram_tensor("fcw", [P, HID2], bf, kind="ExternalInput")
    y_d = nc.dram_tensor("y", [P, WPC], f32, kind="ExternalOutput")

    with tile.TileContext(nc) as tc:
        with (
            tc.tile_pool(name="const", bufs=1) as cpool,
            tc.tile_pool(name="work", bufs=1) as upool,
            tc.tile_pool(name="g", bufs=int(_os.environ.get("KGBUF", "3"))) as gpool,
            tc.tile_pool(name="sel", bufs=int(_os.environ.get("KSBUF", "2"))) as spool,
            tc.tile_pool(name="stage", bufs=2) as stpool,
            tc.tile_pool(name="zc", bufs=2) as zcpool,
            tc.tile_pool(name="tmp", bufs=1) as wpool,
            tc.tile_pool(name="pmm", bufs=2, space="PSUM") as pmm,
            tc.tile_pool(name="pagg", bufs=3, space="PSUM") as pagg,
            tc.tile_pool(name="ptr", bufs=2, space="PSUM") as ptr,
            tc.tile_pool(name="dram", bufs=1, space="DRAM") as dpool,
        ):
            # ---- DRAM scratch ----
            tab1_d = dpool.tile([SPC, TBW], bf)
            tab2_d = dpool.tile([SPC, TBW], bf)
            acc1_d = dpool.tile([NSLOT, HID], bf)
            acc2_d = dpool.tile([NSLOT, HID2], bf)
            rs1_d = dpool.tile([SPC, HID], bf)
            rs2_d = dpool.tile([SPC, HID2], bf)

            # ---- constants ----
            gidx_t = cpool.tile([P, ntiles * 8], i16)
            nc.scalar.dma_start(out=gidx_t[:], in_=gidx_d[:])
            dloc_t = cpool.tile([P, ntiles], bf)
            nc.scalar.dma_start(out=dloc_t[:], in_=dloc_d[:])
            dinv_t = cpool.tile([P, WPC], bf)
            nc.scalar.dma_start(out=dinv_t[:], in_=dinv_d[:])
            iota_t = cpool.tile([P, P], bf)
            nc.scalar.dma_start(out=iota_t[:], in_=iota_d[:])
            w1_t = cpool.tile([IN_CH, HID], bf)
            nc.scalar.dma_start(out=w1_t[:], in_=w1_d[:])
            w2_t = cpool.tile([HID, HID2], bf)
            nc.scalar.dma_start(out=w2_t[:], in_=w2_d[:])
            t1_t = cpool.tile([P, HID], bf)
            nc.scalar.dma_start(out=t1_t[:], in_=t1_d[:])
            t2_t = cpool.tile([P, HID2], bf)
            nc.scalar.dma_start(out=t2_t[:], in_=t2_d[:])
            fcw_t = cpool.tile([P, HID2], bf)
            nc.scalar.dma_start(out=fcw_t[:], in_=fcw_d[:])
            ident = cpool.tile([P, P], bf)
            make_identity(nc, ident[:])

            dinv2_t = cpool.tile([P, WPC], bf)
            nc.vector.tensor_tensor(out=dinv2_t[:], in0=dinv_t[:], in1=dinv_t[:],
                                    op=mybir.AluOpType.mult)

            # ---- L1 dense: u1 = (dinv*x)^T tiles @ W1p (xT in 2 chunks) ----
            u1_t = upool.tile([P, WPC * HID], bf, tag="u1")
            HWPC = WPC // 2
            for half in range(2):
                xc = stpool.tile([P, HWPC * P], bf, tag="stg64",
                                 name=f"xc{half}")
                nc.sync.dma_start(out=xc[:],
                                  in_=xT_d[:, half * HWPC * P:
                                           (half + 1) * HWPC * P])
                for b in range((HWPC + 7) // 8):
                    pm = pmm.tile([P, 512], f32, space="PSUM", tag="pm")
                    ts = range(b * 8, min((b + 1) * 8, HWPC))
                    for i, t in enumerate(ts):
                        nc.tensor.matmul(out=pm[:, i * HID:(i + 1) * HID],
                                         lhsT=xc[:, t * P:(t + 1) * P],
                                         rhs=w1_t[:], start=True, stop=True)
                    nts = len(ts)
                    t0 = half * HWPC + b * 8
                    nc.vector.tensor_copy(
                        out=u1_t[:, t0 * HID:(t0 + nts) * HID],
                        in_=pm[:, 0:nts * HID])

            # table1 rows ell = p*WPC+w (strided 256B)
            nc.sync.dma_start(
                out=bassm.AP(tensor=tab1_d[:].tensor, offset=0,
                             ap=[[WPC * TBW, P], [TBW, WPC], [1, HID]]),
                in_=u1_t[:].rearrange("p (w f) -> p w f", f=HID),
            )

            # ---- per-edge stream: gather + one-hot matmul aggregation ----
            def edge_stream(tab, acc, fw, nwg, layer):
                """Gather dest-sorted messages, build one-hot S tiles on DVE,
                accumulate per-window sums in PSUM on the PE, stage each dest
                core's chunk in SBUF (Act copy) and write it contiguously."""
                cur_pm = [None]
                cur_stg = [None]

                def gslices(W):
                    c_, wi = W // WPC, W % WPC
                    g0 = (wi // nwg) * nwg
                    return c_, wi, g0, min(g0 + nwg, WPC)

                for (t_lo, nt, w_lo, nw) in bins:
                    gv = gpool.tile([P, NTBIN * HID], bf, tag="gv",
                                    name=f"gv{layer}_{t_lo}")
                    gvv = gv[:].rearrange("p (t f) -> p t f", f=fw)[:, 0:nt, :]
                    _dma_gather_raw(
                        nc.gpsimd, nc, gvv,
                        bassm.AP(tensor=tab[:].tensor, offset=0,
                                 ap=[[TBW, SPC], [1, fw]]),
                        gidx_t[:, t_lo * 8:(t_lo + nt) * 8], nt * P, fw, TBW,
                        single_packet=False, queue_num=0)
                    st = spool.tile([P, NTBIN * P], bf, tag="s",
                                    name=f"s{layer}_{t_lo}")
                    st3 = st[:].rearrange("p (t j) -> p t j", j=P)[:, 0:nt, :]
                    nc.vector.tensor_tensor(
                        out=st3,
                        in0=dloc_t[:, t_lo:t_lo + nt, None].to_broadcast([P, nt, P]),
                        in1=iota_t[:, None, :].to_broadcast([P, nt, P]),
                        op=mybir.AluOpType.is_equal)
                    for W in range(w_lo, w_lo + nw):
                        c_, wi, g0, g1_ = gslices(W)
                        if wi == 0:
                            cur_stg[0] = stpool.tile([P, WPC * fw], bf,
                                                     tag="stg64",
                                                     name=f"stg{layer}_{c_}")
                        if wi == g0:
                            cur_pm[0] = pagg.tile([P, 512], f32, space="PSUM",
                                                  tag="agg",
                                                  name=f"agg{layer}_{W}")
                        wrel = wi - g0
                        for k in range(T[W]):
                            trel = win_tile0[W] - t_lo + k
                            nc.tensor.matmul(
                                out=cur_pm[0][:, wrel * fw:(wrel + 1) * fw],
                                lhsT=st[:, trel * P:(trel + 1) * P],
                                rhs=gv[:, trel * fw:(trel + 1) * fw],
                                start=(k == 0), stop=(k == T[W] - 1))
                        if wi == g1_ - 1:
                            nwv = g1_ - g0
                            nc.scalar.activation(
                                out=cur_stg[0][:, g0 * fw:g1_ * fw],
                                in_=cur_pm[0][:, 0:nwv * fw],
                                func=mybir.ActivationFunctionType.Copy)
                        if wi == WPC - 1:
                            nc.sync.dma_start(
                                out=bassm.AP(tensor=acc[:].tensor,
                                             offset=c_ * SPC * fw,
                                             ap=[[WPC * fw, P], [1, WPC * fw]]),
                                in_=cur_stg[0][:])

            edge_stream(tab1_d, acc1_d, HID, 8, 1)

            # ---- RS1 + post1: z = relu(dinv*agg + dinv*u1 + T1) ----
            nc.gpsimd.collective_compute(
                "ReduceScatter", mybir.AluOpType.add,
                replica_groups=[list(range(NCORES))],
                ins=[acc1_d[:]], outs=[rs1_d[:]],
            )

            def precompute_self(u, dvt, tt, fw, tag):
                pre = wpool.tile([P, WPC, fw], bf, tag=f"pre{tag}")
                u3 = u[:].rearrange("p (w f) -> p w f", f=fw)
                nc.vector.tensor_tensor(
                    out=pre[:], in0=u3,
                    in1=dvt[:, :, None].to_broadcast([P, WPC, fw]),
                    op=mybir.AluOpType.mult)
                nc.vector.tensor_tensor(
                    out=pre[:], in0=pre[:],
                    in1=tt[:, None, :].to_broadcast([P, WPC, fw]),
                    op=mybir.AluOpType.add)
                return pre

            # u1 is pre-scaled by dinv, so its self-term multiplier is dinv.
            pre1 = precompute_self(u1_t, dinv_t, t1_t, HID, "1")

            agg1 = upool.tile([P, WPC * HID], bf, tag="agg1")
            nc.sync.dma_start(
                out=agg1[:].rearrange("p (w f) -> p w f", f=HID),
                in_=bassm.AP(tensor=rs1_d[:].tensor, offset=0,
                             ap=[[WPC * HID, P], [HID, WPC], [1, HID]]),
            )

            def post(agg, pre, dvt, fw, out_t):
                a3 = agg[:].rearrange("p (w f) -> p w f", f=fw)
                H0 = WPC // 2
                for h, (lo, nt) in enumerate(((0, H0), (H0, WPC - H0))):
                    tmp = wpool.tile([P, nt, fw], bf, tag=f"pa{fw}h{h}")
                    nc.vector.tensor_tensor(
                        out=tmp[:], in0=a3[:, lo:lo + nt, :],
                        in1=dvt[:, lo:lo + nt, None].to_broadcast([P, nt, fw]),
                        op=mybir.AluOpType.mult)
                    nc.vector.tensor_tensor(out=tmp[:], in0=tmp[:],
                                            in1=pre[:, lo:lo + nt, :],
                                            op=mybir.AluOpType.add)
                    nc.scalar.activation(
                        out=out_t[:, lo * fw:(lo + nt) * fw],
                        in_=tmp[:].rearrange("p t f -> p (t f)"),
                        func=mybir.ActivationFunctionType.Relu)

            z_t = upool.tile([P, WPC * HID], bf, tag="z")
            post(agg1, pre1, dinv_t, HID, z_t)

            # ---- L2 dense: u2 = z @ W2p (PE transpose in 4-tile chunks) ----
            u2_t = upool.tile([P, WPC * HID2], bf, tag="u2")
            t2v = wpool.tile([P, WPC, HID2], bf, tag="t2v")
            tab2_ap = bassm.AP(tensor=tab2_d[:].tensor, offset=0,
                               ap=[[WPC * TBW, P], [TBW, WPC], [1, HID2]])
            for b in range((WPC + 15) // 16):
                pm = pmm.tile([P, 512], f32, space="PSUM", tag="pm", name=f"pm2_{b}")
                ts = list(range(b * 16, min((b + 1) * 16, WPC)))
                for s4 in range(0, len(ts), 4):
                    sub = ts[s4:s4 + 4]
                    tr = ptr.tile([HID, 512], bf, space="PSUM", tag="tr")
                    for i, t in enumerate(sub):
                        nc.tensor.transpose(out=tr[:, i * P:(i + 1) * P],
                                            in_=z_t[:, t * HID:(t + 1) * HID],
                                            identity=ident[:])
                    zc = zcpool.tile([HID, 512], bf, tag="zc",
                                     name=f"zc{b}_{s4}")
                    nc.vector.tensor_copy(out=zc[:, 0:len(sub) * P],
                                          in_=tr[:, 0:len(sub) * P])
                    for i, t in enumerate(sub):
                        nc.tensor.matmul(
                            out=pm[:, (s4 + i) * HID2:(s4 + i + 1) * HID2],
                            lhsT=zc[:, i * P:(i + 1) * P],
                            rhs=w2_t[:], start=True, stop=True)
                nts = len(ts)
                b0 = b * 16
                nc.vector.tensor_copy(out=u2_t[:, b0 * HID2:(b0 + nts) * HID2],
                                      in_=pm[:, 0:nts * HID2])
                nc.vector.tensor_tensor(
                    out=t2v[:, b0:b0 + nts, :],
                    in0=pm[:, 0:nts * HID2].rearrange("p (t f) -> p t f", f=HID2),
                    in1=dinv_t[:, b0:b0 + nts, None].to_broadcast([P, nts, HID2]),
                    op=mybir.AluOpType.mult)
            nc.sync.dma_start(out=tab2_ap, in_=t2v[:])

            edge_stream(tab2_d, acc2_d, HID2, 16, 2)

            # ---- RS2 + post2 + fc ----
            nc.gpsimd.collective_compute(
                "ReduceScatter", mybir.AluOpType.add,
                replica_groups=[list(range(NCORES))],
                ins=[acc2_d[:]], outs=[rs2_d[:]],
            )
            # table2 values dinv*u2 give self term dinv2*u2; u2 is unscaled.
            pre2 = precompute_self(u2_t, dinv2_t, t2_t, HID2, "2")
            agg2 = upool.tile([P, WPC * HID2], bf, tag="agg2")
            nc.sync.dma_start(
                out=agg2[:].rearrange("p (w f) -> p w f", f=HID2),
                in_=bassm.AP(tensor=rs2_d[:].tensor, offset=0,
                             ap=[[WPC * HID2, P], [HID2, WPC], [1, HID2]]),
            )
            h2_t = upool.tile([P, WPC * HID2], bf, tag="h2")
            post(agg2, pre2, dinv_t, HID2, h2_t)

            # fc: y = sum_f h2 * fcW
            prod = wpool.tile([P, WPC, HID2], bf, tag="prod")
            nc.vector.tensor_tensor(
                out=prod[:], in0=h2_t[:].rearrange("p (w f) -> p w f", f=HID2),
                in1=fcw_t[:, None, :].to_broadcast([P, WPC, HID2]),
                op=mybir.AluOpType.mult)
            out_t = upool.tile([P, WPC], f32, tag="out")
            nc.vector.reduce_sum(out=out_t[:, :, None], in_=prod[:],
                                 axis=mybir.AxisListType.X)
            nc.sync.dma_start(out=y_d[:], in_=out_t[:])

    nc.compile()
    return nc


# ----------------------------------------------------------------------
# entry points
# ----------------------------------------------------------------------
def prepare(inputs):
    inputs = {k: np.asarray(v) for k, v in inputs.items()}
    in_maps, consts = host_prep(**inputs)
    nc = build_bass(consts["T"], consts["win_tile0"], consts["bins"],
                    consts["ntiles"])
    return nc, in_maps, consts


def execute(nc, in_maps):
    from concourse.bass_utils import run_bass_kernel_spmd
    return run_bass_kernel_spmd(nc, in_maps, core_ids=list(range(NCORES)))


def unshard(res, consts):
    y = np.zeros((N_NODES, 1), np.float32)
    fcb = consts["fcb"]
    nos = consts["node_of_slot"]
    for c in range(NCORES):
        v = np.asarray(res.results[c]["y"], np.float32).reshape(-1)  # ell order
        valid = nos[c] >= 0
        y[nos[c][valid], 0] = v[valid] + fcb
    return y


def kernel(**inputs):
    nc, in_maps, consts = prepare(inputs)
    res = execute(nc, in_maps)
    return unshard(res, consts)


# revision 14
# speedup vs baseline: 1.4448x; 1.1952x over previous
"""Distributed 2-layer GCN (BangaloreGCN) on 8 Trainium2 NeuronCores.

Matmul-aggregation design (v3):
  * Source-partitioned: core c owns nodes [c*6250, (c+1)*6250) and the
    edges whose SOURCE it owns.  Per layer, each core computes a local
    message table (dinv-scaled dense transform of its own nodes), then
    gathers per-edge messages with dma_gather in DEST-SORTED order.
  * The scatter side is done on the PE array instead of dma_scatter_add:
    the global dest space is split into 400 windows of 128 slots.  Each
    128-edge gathered tile (edge i -> partition i%128) is multiplied by
    a one-hot "selection" matrix S [128 edges, 128 dests] built on the
    DVE (is_equal of per-edge dest-column vs an iota row), accumulating
    partial sums for a window directly in PSUM.  This removes the
    scatter DMA, the accumulator zeroing, and the scatter descriptor
    generation of v2 entirely.
  * Node -> slot assignment is chosen by a greedy packer so that every
    window needs at most 2 tiles per source core (max in-window edge
    count <= 256 for all 8 cores): the SPMD-static stream is ~103k
    indices per core vs 800k/8 = 100k real edges.
  * Slot labeling ell = p*50 + w makes the per-core accumulator chunk
    contiguous per partition, so the PSUM->DRAM staging writes run at
    full DMA rate, and the ReduceScatter chunk c is exactly core c's
    own slots.  Both layers share the identical edge stream, gather
    indices, and S structure (S is rebuilt per layer; it does not fit
    in SBUF).
"""

import sys

sys.path.insert(0, "/opt/trn_rl_repo")

import ml_dtypes
import numpy as np

F16 = np.float16

# ---- problem constants ----
N_NODES = 50000
IN_CH = 128
HID = 64
HID2 = 32
BN_EPS = 1e-5

NCORES = 8
P = 128
WPC = 50                   # windows (tiles) per core
SPC = P * WPC              # 6400 slots per core
NSLOT = NCORES * SPC       # 51200
NWIN = NCORES * WPC        # 400 global windows
REAL = N_NODES // NCORES   # 6250 real nodes per core
WCAP = 127                 # real nodes per window (p=127 spare everywhere)
TBW = 128                  # table row width in bf16 elems (256B stride)
SPARE_ROW = WCAP * WPC     # a slot that is spare on every core (p=127,w=0)
NTBIN = int(__import__("os").environ.get("KNTBIN", "48"))  # tiles per gather bin


# ----------------------------------------------------------------------
# host-side preparation
# ----------------------------------------------------------------------
def _wrap_idx(arr):
    """[n] int -> [128, n/16] int16 image (16-partition wrap, replicated)."""
    ni = arr.shape[0]
    assert ni % 16 == 0
    blk = arr.reshape(ni // 16, 16).T.astype(np.int16)
    return np.tile(blk, (8, 1))


def _pack_windows(Mi):
    """Greedy: assign nodes (rows of Mi [REAL, 8] = per-source-core indeg)
    to WPC windows, minimizing the max per-core in-window load, capped at
    WCAP nodes per window.  Returns win[i] for nodes in degree-sorted
    order and that order."""
    srt = np.argsort(-Mi.sum(1), kind="stable")
    Ms = Mi[srt]
    loads = np.zeros((WPC, NCORES), np.int64)
    cnt = np.zeros(WPC, np.int64)
    win = np.empty(REAL, np.int64)
    big = 1 << 40
    for i in range(REAL):
        cand = (loads + Ms[i]).max(1) + (cnt >= WCAP) * big
        w = int(np.argmin(cand))
        win[i] = w
        loads[w] += Ms[i]
        cnt[w] += 1
    return srt, win


def host_prep(x, edge_index, W1, b1, W2, b2, fcW, fcb,
              g1, be1, rm1, rv1, g2, be2, rm2, rv2):
    row = np.asarray(edge_index[0], np.int64)
    col = np.asarray(edge_index[1], np.int64)
    x = np.asarray(x, np.float32)

    deg = np.bincount(col, minlength=N_NODES).astype(np.float32) + 1.0
    dinv = (1.0 / np.sqrt(deg)).astype(np.float32)

    owner_src = row // REAL

    # ---- node -> slot assignment (window packing per dest core) ----
    M = np.zeros((N_NODES, NCORES), np.int32)
    np.add.at(M, (col, owner_src), 1)
    slot_of_node = np.full(N_NODES, -1, np.int64)      # global slot
    node_of_slot = np.full((NCORES, SPC), -1, np.int64)
    for c in range(NCORES):
        nodes = np.arange(c * REAL, (c + 1) * REAL)
        srt, win = _pack_windows(M[nodes])
        # p = rank within window (stable in assignment order)
        o2 = np.argsort(win, kind="stable")
        wsort = win[o2]
        first = np.zeros(REAL, np.int64)
        starts = np.r_[0, np.flatnonzero(np.diff(wsort)) + 1]
        first[starts] = starts
        first = np.maximum.accumulate(first)
        p_of = np.empty(REAL, np.int64)
        p_of[o2] = np.arange(REAL) - first
        assert p_of.max() < WCAP
        ell = p_of * WPC + win
        slot_of_node[nodes[srt]] = c * SPC + ell
        node_of_slot[c, ell] = nodes[srt]

    # ---- per-edge window/column/source-row ----
    gdst = slot_of_node[col]
    c_d, ell_d = gdst // SPC, gdst % SPC
    p_d, w_d = ell_d // WPC, ell_d % WPC
    gwin = c_d * WPC + w_d                             # [E] global window
    src_slot = slot_of_node[row] % SPC                 # local table row

    # ---- per-window tile counts (static, max over source cores) ----
    cnts = np.zeros((NWIN, NCORES), np.int64)
    np.add.at(cnts, (gwin, owner_src), 1)
    T = np.maximum(1, -(-cnts.max(1) // P)).astype(np.int64)   # [NWIN]
    win_tile0 = np.r_[0, np.cumsum(T)][:-1]
    ntiles = int(T.sum())
    stream = ntiles * P

    # ---- per-core gather index + dest-column streams ----
    gidx_s = np.full((NCORES, stream), SPARE_ROW, np.int64)
    dloc_s = np.full((NCORES, stream), 255, np.int64)
    for h in range(NCORES):
        sel = owner_src == h
        gw, ss, pd = gwin[sel], src_slot[sel], p_d[sel]
        o = np.argsort(gw, kind="stable")
        gw, ss, pd = gw[o], ss[o], pd[o]
        starts = np.r_[0, np.flatnonzero(np.diff(gw)) + 1]
        first = np.zeros(len(gw), np.int64)
        first[starts] = starts
        first = np.maximum.accumulate(first)
        rank = np.arange(len(gw)) - first
        pos = win_tile0[gw] * P + rank
        gidx_s[h, pos] = ss
        dloc_s[h, pos] = pd

    # ---- gather bins: consecutive whole windows, <= NTBIN tiles ----
    bins = []                                          # (t_lo, nt, w_lo, nw)
    w_lo, t_lo = 0, 0
    for W in range(NWIN):
        if W > w_lo and (win_tile0[W] + T[W] - t_lo) > NTBIN:
            bins.append((t_lo, int(win_tile0[W] - t_lo), w_lo, W - w_lo))
            w_lo, t_lo = W, int(win_tile0[W])
    bins.append((t_lo, ntiles - t_lo, w_lo, NWIN - w_lo))
    assert max(b[1] for b in bins) <= NTBIN

    # ---- BN folding ----
    S1c = (np.asarray(g1) / np.sqrt(np.asarray(rv1) + BN_EPS)).astype(np.float32)
    T1 = ((np.asarray(b1) - np.asarray(rm1)) * S1c + np.asarray(be1)).astype(np.float32)
    S2c = (np.asarray(g2) / np.sqrt(np.asarray(rv2) + BN_EPS)).astype(np.float32)
    T2 = ((np.asarray(b2) - np.asarray(rm2)) * S2c + np.asarray(be2)).astype(np.float32)
    W1p = (np.asarray(W1) * S1c[None, :]).astype(np.float32)
    W2p = (np.asarray(W2) * S2c[None, :]).astype(np.float32)

    # ---- per-core tensors ----
    # xT column j holds slot (j%128)*WPC + j//128 so dense tile t yields
    # u1[p, t*HID:..] = slot p*WPC + t.
    colperm = (np.arange(SPC) % P) * WPC + (np.arange(SPC) // P)
    iota = np.tile(np.arange(P, dtype=np.float32)[None, :], (P, 1))
    in_maps = []
    for c in range(NCORES):
        xs = np.zeros((SPC, IN_CH), np.float32)
        dv = np.zeros(SPC, np.float32)
        valid = node_of_slot[c] >= 0
        nd = node_of_slot[c][valid]
        xs[valid] = x[nd] * dinv[nd, None]
        dv[valid] = dinv[nd]
        xs = xs[colperm]                                # [SPC(col j), IN_CH]
        dv_im = dv.reshape(P, WPC)                      # [p, w]
        in_maps.append({
            "xT": np.ascontiguousarray(xs.T).astype(F16),
            "gidx": _wrap_idx(gidx_s[c]),
            "dloc": np.ascontiguousarray(
                dloc_s[c].reshape(ntiles, P).T).astype(F16),
            "dinv": dv_im.astype(F16),
            "iota": iota.astype(F16),
            "w1": W1p.astype(F16),
            "w2": W2p.astype(F16),
            "t1": np.tile(T1[None, :], (P, 1)).astype(F16),
            "t2": np.tile(T2[None, :], (P, 1)).astype(F16),
            "fcw": np.tile(np.asarray(fcW, np.float32).reshape(1, -1),
                           (P, 1)).astype(F16),
        })

    consts = dict(T=T.tolist(), win_tile0=win_tile0.tolist(), bins=bins,
                  ntiles=ntiles, node_of_slot=node_of_slot,
                  fcb=float(np.asarray(fcb).reshape(-1)[0]))
    return in_maps, consts


# ----------------------------------------------------------------------
# raw dma_gather (elem_size below 256B; stride multiple of 256B)
# ----------------------------------------------------------------------
def _dma_gather_raw(gp, bassmod, out_ap, in_ap, idxs_ap, num_idxs, elem_size,
                    elem_step, single_packet=True, queue_num=0):
    import concourse.mybir as mybir
    from concourse import ap_utils
    from concourse.bass import MemorySpace, exact_div, round_up_to_multiple

    assert idxs_ap.dtype == mybir.dt.int16
    assert in_ap.dtype == out_ap.dtype
    assert in_ap.space == MemorySpace.DRAM
    assert idxs_ap.space == MemorySpace.SBUF and out_ap.space == MemorySpace.SBUF
    assert ap_utils.ap_is_contiguous(out_ap.ap[1:])
    assert ap_utils.ap_is_contiguous(idxs_ap.ap[1:])
    assert in_ap.ap[-1][1] == out_ap.ap[-1][1] == elem_size
    assert out_ap.ap[0][1] * out_ap.ap[1][1] == round_up_to_multiple(num_idxs, 128)
    assert in_ap.ap[0][0] == elem_step
    stride_bytes_256 = exact_div(elem_step * mybir.dt.size(in_ap.dtype), 256)
    assert stride_bytes_256 < 256
    return gp.add_instruction(
        mybir.InstDMAGatherAnt(
            name=bassmod.get_next_instruction_name(),
            ins=[*gp.lower_ap_dma(in_ap, for_custom_bir_dma=True),
                 gp.lower_ap(idxs_ap),
                 gp.lower_val_access(gp.to_reg(num_idxs))],
            outs=[gp.lower_ap(out_ap)],
            transpose=False,
            num_idxs=num_idxs,
            elem_size=elem_size,
            stride_bytes_256=stride_bytes_256,
            gen_mode=0,
            single_packet=single_packet,
            queue_num=queue_num,
            sbuf_tokens_per_rank=0,
            sbuf_free_dim_per_rank=0,
            sbuf_free_dim_pad_per_rank=0,
            sbuf_byte_offset=0,
        ))


# ----------------------------------------------------------------------
# device program
# ----------------------------------------------------------------------
def build_bass(T, win_tile0, bins, ntiles):
    import concourse.bacc as bacc
    import concourse.bass as bassm
    import concourse.mybir as mybir
    import concourse.tile as tile
    from concourse.masks import make_identity

    f32 = mybir.dt.float32
    bf = mybir.dt.float16
    i16 = mybir.dt.int16

    import os as _os
    nc = bacc.Bacc("TRN2", target_bir_lowering=False,
                   dynamic_dma_scratch_size=int(_os.environ.get("KSCRATCH", "49152")),
                   num_swdge_queues=1)
    xT_d = nc.dram_tensor("xT", [P, SPC], bf, kind="ExternalInput")
    gidx_d = nc.dram_tensor("gidx", [P, ntiles * 8], i16, kind="ExternalInput")
    dloc_d = nc.dram_tensor("dloc", [P, ntiles], bf, kind="ExternalInput")
    dinv_d = nc.dram_tensor("dinv", [P, WPC], bf, kind="ExternalInput")
    iota_d = nc.dram_tensor("iota", [P, P], bf, kind="ExternalInput")
    w1_d = nc.dram_tensor("w1", [IN_CH, HID], bf, kind="ExternalInput")
    w2_d = nc.dram_tensor("w2", [HID, HID2], bf, kind="ExternalInput")
    t1_d = nc.dram_tensor("t1", [P, HID], bf, kind="ExternalInput")
    t2_d = nc.dram_tensor("t2", [P, HID2], bf, kind="ExternalInput")
    fcw_d = nc.dram_tensor("fcw", [P, HID2], bf, kind="ExternalInput")
    y_d = nc.dram_tensor("y", [P, WPC], f32, kind="ExternalOutput")

    with tile.TileContext(nc) as tc:
        with (
            tc.tile_pool(name="const", bufs=1) as cpool,
            tc.tile_pool(name="work", bufs=1) as upool,
            tc.tile_pool(name="g", bufs=int(_os.environ.get("KGBUF", "3"))) as gpool,
            tc.tile_pool(name="sel", bufs=int(_os.environ.get("KSBUF", "2"))) as spool,
            tc.tile_pool(name="stage", bufs=2) as stpool,
            tc.tile_pool(name="zc", bufs=2) as zcpool,
            tc.tile_pool(name="tmp", bufs=1) as wpool,
            tc.tile_pool(name="pmm", bufs=2, space="PSUM") as pmm,
            tc.tile_pool(name="pagg", bufs=3, space="PSUM") as pagg,
            tc.tile_pool(name="ptr", bufs=2, space="PSUM") as ptr,
            tc.tile_pool(name="dram", bufs=1, space="DRAM") as dpool,
        ):
            # ---- DRAM scratch ----
            tab1_d = dpool.tile([SPC, TBW], bf)
            tab2_d = dpool.tile([SPC, TBW], bf)
            acc1_d = dpool.tile([NSLOT, HID], bf)
            acc2_d = dpool.tile([NSLOT, HID2], bf)
            rs1_d = dpool.tile([SPC, HID], bf)
            rs2_d = dpool.tile([SPC, HID2], bf)

            # ---- constants ----
            gidx_t = cpool.tile([P, ntiles * 8], i16)
            nc.scalar.dma_start(out=gidx_t[:], in_=gidx_d[:])
            dloc_t = cpool.tile([P, ntiles], bf)
            nc.scalar.dma_start(out=dloc_t[:], in_=dloc_d[:])
            dinv_t = cpool.tile([P, WPC], bf)
            nc.scalar.dma_start(out=dinv_t[:], in_=dinv_d[:])
            iota_t = cpool.tile([P, P], bf)
            nc.scalar.dma_start(out=iota_t[:], in_=iota_d[:])
            w1_t = cpool.tile([IN_CH, HID], bf)
            nc.scalar.dma_start(out=w1_t[:], in_=w1_d[:])
            w2_t = cpool.tile([HID, HID2], bf)
            nc.scalar.dma_start(out=w2_t[:], in_=w2_d[:])
            t1_t = cpool.tile([P, HID], bf)
            nc.scalar.dma_start(out=t1_t[:], in_=t1_d[:])
            t2_t = cpool.tile([P, HID2], bf)
            nc.scalar.dma_start(out=t2_t[:], in_=t2_d[:])
            fcw_t = cpool.tile([P, HID2], bf)
            nc.scalar.dma_start(out=fcw_t[:], in_=fcw_d[:])
            ident = cpool.tile([P, P], bf)
            make_identity(nc, ident[:])

            dinv2_t = cpool.tile([P, WPC], bf)
            nc.vector.tensor_tensor(out=dinv2_t[:], in0=dinv_t[:], in1=dinv_t[:],
                                    op=mybir.AluOpType.mult)
            # f32 copy of dloc (tensor_scalar is_equal wants an f32 scalar AP)
            dlocf_t = cpool.tile([P, ntiles], f32)
            nc.vector.tensor_copy(out=dlocf_t[:], in_=dloc_t[:])

            # ---- L1 dense: u1 = (dinv*x)^T tiles @ W1p (xT in 2 chunks) ----
            u1_t = upool.tile([P, WPC * HID], bf, tag="u1")
            HWPC = WPC // 2
            for half in range(2):
                xc = stpool.tile([P, HWPC * P], bf, tag="stg64",
                                 name=f"xc{half}")
                nc.sync.dma_start(out=xc[:],
                                  in_=xT_d[:, half * HWPC * P:
                                           (half + 1) * HWPC * P])
                for b in range((HWPC + 7) // 8):
                    pm = pmm.tile([P, 512], f32, space="PSUM", tag="pm")
                    ts = range(b * 8, min((b + 1) * 8, HWPC))
                    for i, t in enumerate(ts):
                        nc.tensor.matmul(out=pm[:, i * HID:(i + 1) * HID],
                                         lhsT=xc[:, t * P:(t + 1) * P],
                                         rhs=w1_t[:], start=True, stop=True)
                    nts = len(ts)
                    t0 = half * HWPC + b * 8
                    nc.vector.tensor_copy(
                        out=u1_t[:, t0 * HID:(t0 + nts) * HID],
                        in_=pm[:, 0:nts * HID])

            # table1 rows ell = p*WPC+w (strided 256B)
            nc.sync.dma_start(
                out=bassm.AP(tensor=tab1_d[:].tensor, offset=0,
                             ap=[[WPC * TBW, P], [TBW, WPC], [1, HID]]),
                in_=u1_t[:].rearrange("p (w f) -> p w f", f=HID),
            )

            # ---- per-edge stream: gather + one-hot matmul aggregation ----
            def edge_stream(tab, acc, fw, nwg, layer):
                """Gather dest-sorted messages, build one-hot S tiles on DVE,
                accumulate per-window sums in PSUM on the PE, stage each dest
                core's chunk in SBUF (Act copy) and write it contiguously."""
                cur_pm = [None]
                cur_stg = [None]

                def gslices(W):
                    c_, wi = W // WPC, W % WPC
                    g0 = (wi // nwg) * nwg
                    return c_, wi, g0, min(g0 + nwg, WPC)

                for (t_lo, nt, w_lo, nw) in bins:
                    gv = gpool.tile([P, NTBIN * HID], bf, tag="gv",
                                    name=f"gv{layer}_{t_lo}")
                    gvv = gv[:].rearrange("p (t f) -> p t f", f=fw)[:, 0:nt, :]
                    _dma_gather_raw(
                        nc.gpsimd, nc, gvv,
                        bassm.AP(tensor=tab[:].tensor, offset=0,
                                 ap=[[TBW, SPC], [1, fw]]),
                        gidx_t[:, t_lo * 8:(t_lo + nt) * 8], nt * P, fw, TBW,
                        single_packet=False, queue_num=0)
                    st = spool.tile([P, NTBIN * P], bf, tag="s",
                                    name=f"s{layer}_{t_lo}")
                    for trel in range(nt):
                        nc.vector.tensor_scalar(
                            out=st[:, trel * P:(trel + 1) * P],
                            in0=iota_t[:],
                            scalar1=dlocf_t[:, t_lo + trel:t_lo + trel + 1],
                            scalar2=None, op0=mybir.AluOpType.is_equal)
                    for W in range(w_lo, w_lo + nw):
                        c_, wi, g0, g1_ = gslices(W)
                        if wi == 0:
                            cur_stg[0] = stpool.tile([P, WPC * fw], bf,
                                                     tag="stg64",
                                                     name=f"stg{layer}_{c_}")
                        if wi == g0:
                            cur_pm[0] = pagg.tile([P, 512], f32, space="PSUM",
                                                  tag="agg",
                                                  name=f"agg{layer}_{W}")
                        wrel = wi - g0
                        for k in range(T[W]):
                            trel = win_tile0[W] - t_lo + k
                            nc.tensor.matmul(
                                out=cur_pm[0][:, wrel * fw:(wrel + 1) * fw],
                                lhsT=st[:, trel * P:(trel + 1) * P],
                                rhs=gv[:, trel * fw:(trel + 1) * fw],
                                start=(k == 0), stop=(k == T[W] - 1))
                        if wi == g1_ - 1:
                            nwv = g1_ - g0
                            nc.scalar.activation(
                                out=cur_stg[0][:, g0 * fw:g1_ * fw],
                                in_=cur_pm[0][:, 0:nwv * fw],
                                func=mybir.ActivationFunctionType.Copy)
                        if wi == WPC - 1:
                            nc.sync.dma_start(
                                out=bassm.AP(tensor=acc[:].tensor,
                                             offset=c_ * SPC * fw,
                                             ap=[[WPC * fw, P], [1, WPC * fw]]),
                                in_=cur_stg[0][:])

            edge_stream(tab1_d, acc1_d, HID, 8, 1)

            # ---- RS1 + post1: z = relu(dinv*agg + dinv*u1 + T1) ----
            nc.gpsimd.collective_compute(
                "ReduceScatter", mybir.AluOpType.add,
                replica_groups=[list(range(NCORES))],
                ins=[acc1_d[:]], outs=[rs1_d[:]],
            )

            def precompute_self(u, dvt, tt, fw, tag):
                pre = wpool.tile([P, WPC, fw], bf, tag=f"pre{tag}")
                u3 = u[:].rearrange("p (w f) -> p w f", f=fw)
                nc.vector.tensor_tensor(
                    out=pre[:], in0=u3,
                    in1=dvt[:, :, None].to_broadcast([P, WPC, fw]),
                    op=mybir.AluOpType.mult)
                nc.vector.tensor_tensor(
                    out=pre[:], in0=pre[:],
                    in1=tt[:, None, :].to_broadcast([P, WPC, fw]),
                    op=mybir.AluOpType.add)
                return pre

            # u1 is pre-scaled by dinv, so its self-term multiplier is dinv.
            pre1 = precompute_self(u1_t, dinv_t, t1_t, HID, "1")

            agg1 = upool.tile([P, WPC * HID], bf, tag="agg1")
            nc.sync.dma_start(
                out=agg1[:].rearrange("p (w f) -> p w f", f=HID),
                in_=bassm.AP(tensor=rs1_d[:].tensor, offset=0,
                             ap=[[WPC * HID, P], [HID, WPC], [1, HID]]),
            )

            def post(agg, pre, dvt, fw, out_t):
                a3 = agg[:].rearrange("p (w f) -> p w f", f=fw)
                H0 = WPC // 2
                for h, (lo, nt) in enumerate(((0, H0), (H0, WPC - H0))):
                    tmp = wpool.tile([P, nt, fw], bf, tag=f"pa{fw}h{h}")
                    nc.vector.tensor_tensor(
                        out=tmp[:], in0=a3[:, lo:lo + nt, :],
                        in1=dvt[:, lo:lo + nt, None].to_broadcast([P, nt, fw]),
                        op=mybir.AluOpType.mult)
                    nc.vector.tensor_tensor(out=tmp[:], in0=tmp[:],
                                            in1=pre[:, lo:lo + nt, :],
                                            op=mybir.AluOpType.add)
                    nc.scalar.activation(
                        out=out_t[:, lo * fw:(lo + nt) * fw],
                        in_=tmp[:].rearrange("p t f -> p (t f)"),
                        func=mybir.ActivationFunctionType.Relu)

            z_t = upool.tile([P, WPC * HID], bf, tag="z")
            post(agg1, pre1, dinv_t, HID, z_t)

            # ---- L2 dense: u2 = z @ W2p (PE transpose in 4-tile chunks) ----
            u2_t = upool.tile([P, WPC * HID2], bf, tag="u2")
            t2v = wpool.tile([P, WPC, HID2], bf, tag="t2v")
            tab2_ap = bassm.AP(tensor=tab2_d[:].tensor, offset=0,
                               ap=[[WPC * TBW, P], [TBW, WPC], [1, HID2]])
            for b in range((WPC + 15) // 16):
                pm = pmm.tile([P, 512], f32, space="PSUM", tag="pm", name=f"pm2_{b}")
                ts = list(range(b * 16, min((b + 1) * 16, WPC)))
                for s4 in range(0, len(ts), 4):
                    sub = ts[s4:s4 + 4]
                    tr = ptr.tile([HID, 512], bf, space="PSUM", tag="tr")
                    for i, t in enumerate(sub):
                        nc.tensor.transpose(out=tr[:, i * P:(i + 1) * P],
                                            in_=z_t[:, t * HID:(t + 1) * HID],
                                            identity=ident[:])
                    zc = zcpool.tile([HID, 512], bf, tag="zc",
                                     name=f"zc{b}_{s4}")
                    nc.vector.tensor_copy(out=zc[:, 0:len(sub) * P],
                                          in_=tr[:, 0:len(sub) * P])
                    for i, t in enumerate(sub):
                        nc.tensor.matmul(
                            out=pm[:, (s4 + i) * HID2:(s4 + i + 1) * HID2],
                            lhsT=zc[:, i * P:(i + 1) * P],
                            rhs=w2_t[:], start=True, stop=True)
                nts = len(ts)
                b0 = b * 16
                nc.vector.tensor_copy(out=u2_t[:, b0 * HID2:(b0 + nts) * HID2],
                                      in_=pm[:, 0:nts * HID2])
                nc.vector.tensor_tensor(
                    out=t2v[:, b0:b0 + nts, :],
                    in0=pm[:, 0:nts * HID2].rearrange("p (t f) -> p t f", f=HID2),
                    in1=dinv_t[:, b0:b0 + nts, None].to_broadcast([P, nts, HID2]),
                    op=mybir.AluOpType.mult)
            nc.sync.dma_start(out=tab2_ap, in_=t2v[:])

            edge_stream(tab2_d, acc2_d, HID2, 16, 2)

            # ---- RS2 + post2 + fc ----
            nc.gpsimd.collective_compute(
                "ReduceScatter", mybir.AluOpType.add,
                replica_groups=[list(range(NCORES))],
                ins=[acc2_d[:]], outs=[rs2_d[:]],
            )
            # table2 values dinv*u2 give self term dinv2*u2; u2 is unscaled.
            pre2 = precompute_self(u2_t, dinv2_t, t2_t, HID2, "2")
            agg2 = upool.tile([P, WPC * HID2], bf, tag="agg2")
            nc.sync.dma_start(
                out=agg2[:].rearrange("p (w f) -> p w f", f=HID2),
                in_=bassm.AP(tensor=rs2_d[:].tensor, offset=0,
                             ap=[[WPC * HID2, P], [HID2, WPC], [1, HID2]]),
            )
            h2_t = upool.tile([P, WPC * HID2], bf, tag="h2")
            post(agg2, pre2, dinv_t, HID2, h2_t)

            # fc: y = sum_f h2 * fcW
            prod = wpool.tile([P, WPC, HID2], bf, tag="prod")
            nc.vector.tensor_tensor(
                out=prod[:], in0=h2_t[:].rearrange("p (w f) -> p w f", f=HID2),
                in1=fcw_t[:, None, :].to_broadcast([P, WPC, HID2]),
                op=mybir.AluOpType.mult)
            out_t = upool.tile([P, WPC], f32, tag="out")
            nc.vector.reduce_sum(out=out_t[:, :, None], in_=prod[:],
                                 axis=mybir.AxisListType.X)
            nc.sync.dma_start(out=y_d[:], in_=out_t[:])

    nc.compile()
    return nc


# ----------------------------------------------------------------------
# entry points
# ----------------------------------------------------------------------
def prepare(inputs):
    inputs = {k: np.asarray(v) for k, v in inputs.items()}
    in_maps, consts = host_prep(**inputs)
    nc = build_bass(consts["T"], consts["win_tile0"], consts["bins"],
                    consts["ntiles"])
    return nc, in_maps, consts


def execute(nc, in_maps):
    from concourse.bass_utils import run_bass_kernel_spmd
    return run_bass_kernel_spmd(nc, in_maps, core_ids=list(range(NCORES)))


def unshard(res, consts):
    y = np.zeros((N_NODES, 1), np.float32)
    fcb = consts["fcb"]
    nos = consts["node_of_slot"]
    for c in range(NCORES):
        v = np.asarray(res.results[c]["y"], np.float32).reshape(-1)  # ell order
        valid = nos[c] >= 0
        y[nos[c][valid], 0] = v[valid] + fcb
    return y


def kernel(**inputs):
    nc, in_maps, consts = prepare(inputs)
    res = execute(nc, in_maps)
    return unshard(res, consts)


# revision 18
# speedup vs baseline: 1.5200x; 1.0520x over previous
"""Distributed 2-layer GCN (BangaloreGCN) on 8 Trainium2 NeuronCores.

Matmul-aggregation design (v3):
  * Source-partitioned: core c owns nodes [c*6250, (c+1)*6250) and the
    edges whose SOURCE it owns.  Per layer, each core computes a local
    message table (dinv-scaled dense transform of its own nodes), then
    gathers per-edge messages with dma_gather in DEST-SORTED order.
  * The scatter side is done on the PE array instead of dma_scatter_add:
    the global dest space is split into 400 windows of 128 slots.  Each
    128-edge gathered tile (edge i -> partition i%128) is multiplied by
    a one-hot "selection" matrix S [128 edges, 128 dests] built on the
    DVE (is_equal of per-edge dest-column vs an iota row), accumulating
    partial sums for a window directly in PSUM.  This removes the
    scatter DMA, the accumulator zeroing, and the scatter descriptor
    generation of v2 entirely.
  * Node -> slot assignment is chosen by a greedy packer so that every
    window needs at most 2 tiles per source core (max in-window edge
    count <= 256 for all 8 cores): the SPMD-static stream is ~103k
    indices per core vs 800k/8 = 100k real edges.
  * Slot labeling ell = p*50 + w makes the per-core accumulator chunk
    contiguous per partition, so the PSUM->DRAM staging writes run at
    full DMA rate, and the ReduceScatter chunk c is exactly core c's
    own slots.  Both layers share the identical edge stream, gather
    indices, and S structure (S is rebuilt per layer; it does not fit
    in SBUF).
"""

import sys

sys.path.insert(0, "/opt/trn_rl_repo")

import ml_dtypes
import numpy as np

F16 = np.float16

# ---- problem constants ----
N_NODES = 50000
IN_CH = 128
HID = 64
HID2 = 32
BN_EPS = 1e-5

NCORES = 8
P = 128
WPC = 50                   # windows (tiles) per core
SPC = P * WPC              # 6400 slots per core
NSLOT = NCORES * SPC       # 51200
NWIN = NCORES * WPC        # 400 global windows
REAL = N_NODES // NCORES   # 6250 real nodes per core
WCAP = 127                 # real nodes per window (p=127 spare everywhere)
TBW = 128                  # table row width in bf16 elems (256B stride)
SPARE_ROW = WCAP * WPC     # a slot that is spare on every core (p=127,w=0)
NTBIN = int(__import__("os").environ.get("KNTBIN", "48"))  # tiles per gather bin


# ----------------------------------------------------------------------
# host-side preparation
# ----------------------------------------------------------------------
def _wrap_idx(arr):
    """[n] int -> [128, n/16] int16 image (16-partition wrap, replicated)."""
    ni = arr.shape[0]
    assert ni % 16 == 0
    blk = arr.reshape(ni // 16, 16).T.astype(np.int16)
    return np.tile(blk, (8, 1))


def _pack_windows(Mi):
    """Greedy: assign nodes (rows of Mi [REAL, 8] = per-source-core indeg)
    to WPC windows, minimizing the max per-core in-window load, capped at
    WCAP nodes per window.  Returns win[i] for nodes in degree-sorted
    order and that order."""
    srt = np.argsort(-Mi.sum(1), kind="stable")
    Ms = Mi[srt]
    loads = np.zeros((WPC, NCORES), np.int64)
    cnt = np.zeros(WPC, np.int64)
    win = np.empty(REAL, np.int64)
    big = 1 << 40
    for i in range(REAL):
        cand = (loads + Ms[i]).max(1) + (cnt >= WCAP) * big
        w = int(np.argmin(cand))
        win[i] = w
        loads[w] += Ms[i]
        cnt[w] += 1
    return srt, win


def host_prep(x, edge_index, W1, b1, W2, b2, fcW, fcb,
              g1, be1, rm1, rv1, g2, be2, rm2, rv2):
    row = np.asarray(edge_index[0], np.int64)
    col = np.asarray(edge_index[1], np.int64)
    x = np.asarray(x, np.float32)

    deg = np.bincount(col, minlength=N_NODES).astype(np.float32) + 1.0
    dinv = (1.0 / np.sqrt(deg)).astype(np.float32)

    owner_src = row // REAL

    # ---- node -> slot assignment (window packing per dest core) ----
    M = np.zeros((N_NODES, NCORES), np.int32)
    np.add.at(M, (col, owner_src), 1)
    slot_of_node = np.full(N_NODES, -1, np.int64)      # global slot
    node_of_slot = np.full((NCORES, SPC), -1, np.int64)
    for c in range(NCORES):
        nodes = np.arange(c * REAL, (c + 1) * REAL)
        srt, win = _pack_windows(M[nodes])
        # p = rank within window (stable in assignment order)
        o2 = np.argsort(win, kind="stable")
        wsort = win[o2]
        first = np.zeros(REAL, np.int64)
        starts = np.r_[0, np.flatnonzero(np.diff(wsort)) + 1]
        first[starts] = starts
        first = np.maximum.accumulate(first)
        p_of = np.empty(REAL, np.int64)
        p_of[o2] = np.arange(REAL) - first
        assert p_of.max() < WCAP
        ell = p_of * WPC + win
        slot_of_node[nodes[srt]] = c * SPC + ell
        node_of_slot[c, ell] = nodes[srt]

    # ---- per-edge window/column/source-row ----
    # Global window order is HALF-major (h = win // HWPC), then dest core,
    # then window-within-half: the ReduceScatter for half h covers acc rows
    # [h*NSLOT/2, (h+1)*NSLOT/2) and fires as soon as half h's windows are
    # done -- half A's RS overlaps half B's edge stream.
    gdst = slot_of_node[col]
    c_d, ell_d = gdst // SPC, gdst % SPC
    p_d, w_d = ell_d // WPC, ell_d % WPC
    HWPC = WPC // 2
    gwin = (w_d // HWPC) * (NCORES * HWPC) + c_d * HWPC + (w_d % HWPC)
    src_slot = slot_of_node[row] % SPC                 # local table row

    # ---- per-window tile counts (static, max over source cores) ----
    cnts = np.zeros((NWIN, NCORES), np.int64)
    np.add.at(cnts, (gwin, owner_src), 1)
    T = np.maximum(1, -(-cnts.max(1) // P)).astype(np.int64)   # [NWIN]
    win_tile0 = np.r_[0, np.cumsum(T)][:-1]
    ntiles = int(T.sum())
    stream = ntiles * P

    # ---- per-core gather index + dest-column streams ----
    gidx_s = np.full((NCORES, stream), SPARE_ROW, np.int64)
    dloc_s = np.full((NCORES, stream), 255, np.int64)
    for h in range(NCORES):
        sel = owner_src == h
        gw, ss, pd = gwin[sel], src_slot[sel], p_d[sel]
        o = np.argsort(gw, kind="stable")
        gw, ss, pd = gw[o], ss[o], pd[o]
        starts = np.r_[0, np.flatnonzero(np.diff(gw)) + 1]
        first = np.zeros(len(gw), np.int64)
        first[starts] = starts
        first = np.maximum.accumulate(first)
        rank = np.arange(len(gw)) - first
        pos = win_tile0[gw] * P + rank
        gidx_s[h, pos] = ss
        dloc_s[h, pos] = pd

    # ---- gather bins: consecutive whole windows, <= NTBIN tiles;
    #      forced break at the half boundary ----
    bins = []                                          # (t_lo, nt, w_lo, nw)
    w_lo, t_lo = 0, 0
    for W in range(NWIN):
        if W > w_lo and ((win_tile0[W] + T[W] - t_lo) > NTBIN
                         or W == NWIN // 2):
            bins.append((t_lo, int(win_tile0[W] - t_lo), w_lo, W - w_lo))
            w_lo, t_lo = W, int(win_tile0[W])
    bins.append((t_lo, ntiles - t_lo, w_lo, NWIN - w_lo))
    assert max(b[1] for b in bins) <= NTBIN

    # ---- BN folding ----
    S1c = (np.asarray(g1) / np.sqrt(np.asarray(rv1) + BN_EPS)).astype(np.float32)
    T1 = ((np.asarray(b1) - np.asarray(rm1)) * S1c + np.asarray(be1)).astype(np.float32)
    S2c = (np.asarray(g2) / np.sqrt(np.asarray(rv2) + BN_EPS)).astype(np.float32)
    T2 = ((np.asarray(b2) - np.asarray(rm2)) * S2c + np.asarray(be2)).astype(np.float32)
    W1p = (np.asarray(W1) * S1c[None, :]).astype(np.float32)
    W2p = (np.asarray(W2) * S2c[None, :]).astype(np.float32)

    # ---- per-core tensors ----
    # xT column j holds slot (j%128)*WPC + j//128 so dense tile t yields
    # u1[p, t*HID:..] = slot p*WPC + t.
    colperm = (np.arange(SPC) % P) * WPC + (np.arange(SPC) // P)
    iota = np.tile(np.arange(P, dtype=np.float32)[None, :], (P, 1))
    in_maps = []
    for c in range(NCORES):
        xs = np.zeros((SPC, IN_CH), np.float32)
        dv = np.zeros(SPC, np.float32)
        valid = node_of_slot[c] >= 0
        nd = node_of_slot[c][valid]
        xs[valid] = x[nd] * dinv[nd, None]
        dv[valid] = dinv[nd]
        xs = xs[colperm]                                # [SPC(col j), IN_CH]
        dv_im = dv.reshape(P, WPC)                      # [p, w]
        in_maps.append({
            "xT": np.ascontiguousarray(xs.T).astype(F16),
            "gidx": _wrap_idx(gidx_s[c]),
            "dloc": np.ascontiguousarray(
                dloc_s[c].reshape(ntiles, P).T).astype(F16),
            "dinv": dv_im.astype(F16),
            "iota": iota.astype(F16),
            "w1": W1p.astype(F16),
            "w2": W2p.astype(F16),
            "t1": np.tile(T1[None, :], (P, 1)).astype(F16),
            "t2": np.tile(T2[None, :], (P, 1)).astype(F16),
            "fcw": np.tile(np.asarray(fcW, np.float32).reshape(1, -1),
                           (P, 1)).astype(F16),
        })

    consts = dict(T=T.tolist(), win_tile0=win_tile0.tolist(), bins=bins,
                  ntiles=ntiles, node_of_slot=node_of_slot,
                  fcb=float(np.asarray(fcb).reshape(-1)[0]))
    return in_maps, consts


# ----------------------------------------------------------------------
# raw dma_gather (elem_size below 256B; stride multiple of 256B)
# ----------------------------------------------------------------------
def _dma_gather_raw(gp, bassmod, out_ap, in_ap, idxs_ap, num_idxs, elem_size,
                    elem_step, single_packet=True, queue_num=0):
    import concourse.mybir as mybir
    from concourse import ap_utils
    from concourse.bass import MemorySpace, exact_div, round_up_to_multiple

    assert idxs_ap.dtype == mybir.dt.int16
    assert in_ap.dtype == out_ap.dtype
    assert in_ap.space == MemorySpace.DRAM
    assert idxs_ap.space == MemorySpace.SBUF and out_ap.space == MemorySpace.SBUF
    assert ap_utils.ap_is_contiguous(out_ap.ap[1:])
    assert ap_utils.ap_is_contiguous(idxs_ap.ap[1:])
    assert in_ap.ap[-1][1] == out_ap.ap[-1][1] == elem_size
    assert out_ap.ap[0][1] * out_ap.ap[1][1] == round_up_to_multiple(num_idxs, 128)
    assert in_ap.ap[0][0] == elem_step
    stride_bytes_256 = exact_div(elem_step * mybir.dt.size(in_ap.dtype), 256)
    assert stride_bytes_256 < 256
    return gp.add_instruction(
        mybir.InstDMAGatherAnt(
            name=bassmod.get_next_instruction_name(),
            ins=[*gp.lower_ap_dma(in_ap, for_custom_bir_dma=True),
                 gp.lower_ap(idxs_ap),
                 gp.lower_val_access(gp.to_reg(num_idxs))],
            outs=[gp.lower_ap(out_ap)],
            transpose=False,
            num_idxs=num_idxs,
            elem_size=elem_size,
            stride_bytes_256=stride_bytes_256,
            gen_mode=0,
            single_packet=single_packet,
            queue_num=queue_num,
            sbuf_tokens_per_rank=0,
            sbuf_free_dim_per_rank=0,
            sbuf_free_dim_pad_per_rank=0,
            sbuf_byte_offset=0,
        ))


# ----------------------------------------------------------------------
# device program
# ----------------------------------------------------------------------
def build_bass(T, win_tile0, bins, ntiles):
    import concourse.bacc as bacc
    import concourse.bass as bassm
    import concourse.mybir as mybir
    import concourse.tile as tile
    from concourse.masks import make_identity

    f32 = mybir.dt.float32
    bf = mybir.dt.float16
    i16 = mybir.dt.int16

    import os as _os
    nc = bacc.Bacc("TRN2", target_bir_lowering=False,
                   dynamic_dma_scratch_size=int(_os.environ.get("KSCRATCH", "49152")),
                   num_swdge_queues=1)
    xT_d = nc.dram_tensor("xT", [P, SPC], bf, kind="ExternalInput")
    gidx_d = nc.dram_tensor("gidx", [P, ntiles * 8], i16, kind="ExternalInput")
    dloc_d = nc.dram_tensor("dloc", [P, ntiles], bf, kind="ExternalInput")
    dinv_d = nc.dram_tensor("dinv", [P, WPC], bf, kind="ExternalInput")
    iota_d = nc.dram_tensor("iota", [P, P], bf, kind="ExternalInput")
    w1_d = nc.dram_tensor("w1", [IN_CH, HID], bf, kind="ExternalInput")
    w2_d = nc.dram_tensor("w2", [HID, HID2], bf, kind="ExternalInput")
    t1_d = nc.dram_tensor("t1", [P, HID], bf, kind="ExternalInput")
    t2_d = nc.dram_tensor("t2", [P, HID2], bf, kind="ExternalInput")
    fcw_d = nc.dram_tensor("fcw", [P, HID2], bf, kind="ExternalInput")
    y_d = nc.dram_tensor("y", [P, WPC], f32, kind="ExternalOutput")

    with tile.TileContext(nc) as tc:
        with (
            tc.tile_pool(name="const", bufs=1) as cpool,
            tc.tile_pool(name="work", bufs=1) as upool,
            tc.tile_pool(name="g", bufs=int(_os.environ.get("KGBUF", "3"))) as gpool,
            tc.tile_pool(name="sel", bufs=int(_os.environ.get("KSBUF", "2"))) as spool,
            tc.tile_pool(name="stage", bufs=2) as stpool,
            tc.tile_pool(name="zc", bufs=2) as zcpool,
            tc.tile_pool(name="tmp", bufs=1) as wpool,
            tc.tile_pool(name="pmm", bufs=2, space="PSUM") as pmm,
            tc.tile_pool(name="pagg", bufs=3, space="PSUM") as pagg,
            tc.tile_pool(name="ptr", bufs=2, space="PSUM") as ptr,
            tc.tile_pool(name="dram", bufs=1, space="DRAM") as dpool,
        ):
            # ---- DRAM scratch ----
            tab1_d = dpool.tile([SPC, TBW], bf)
            tab2_d = dpool.tile([SPC, TBW], bf)
            acc1_d = dpool.tile([NSLOT, HID], bf)
            acc2_d = dpool.tile([NSLOT, HID2], bf)
            rs1_d = dpool.tile([SPC, HID], bf)
            rs2_d = dpool.tile([SPC, HID2], bf)

            # ---- constants ----
            gidx_t = cpool.tile([P, ntiles * 8], i16)
            nc.scalar.dma_start(out=gidx_t[:], in_=gidx_d[:])
            dloc_t = cpool.tile([P, ntiles], bf)
            nc.scalar.dma_start(out=dloc_t[:], in_=dloc_d[:])
            dinv_t = cpool.tile([P, WPC], bf)
            nc.scalar.dma_start(out=dinv_t[:], in_=dinv_d[:])
            iota_t = cpool.tile([P, P], bf)
            nc.scalar.dma_start(out=iota_t[:], in_=iota_d[:])
            w1_t = cpool.tile([IN_CH, HID], bf)
            nc.scalar.dma_start(out=w1_t[:], in_=w1_d[:])
            w2_t = cpool.tile([HID, HID2], bf)
            nc.scalar.dma_start(out=w2_t[:], in_=w2_d[:])
            t1_t = cpool.tile([P, HID], bf)
            nc.scalar.dma_start(out=t1_t[:], in_=t1_d[:])
            t2_t = cpool.tile([P, HID2], bf)
            nc.scalar.dma_start(out=t2_t[:], in_=t2_d[:])
            fcw_t = cpool.tile([P, HID2], bf)
            nc.scalar.dma_start(out=fcw_t[:], in_=fcw_d[:])
            ident = cpool.tile([P, P], bf)
            make_identity(nc, ident[:])

            dinv2_t = cpool.tile([P, WPC], bf)
            nc.vector.tensor_tensor(out=dinv2_t[:], in0=dinv_t[:], in1=dinv_t[:],
                                    op=mybir.AluOpType.mult)
            # f32 copy of dloc (tensor_scalar is_equal wants an f32 scalar AP)
            dlocf_t = cpool.tile([P, ntiles], f32)
            nc.vector.tensor_copy(out=dlocf_t[:], in_=dloc_t[:])

            # ---- L1 dense: u1 = (dinv*x)^T tiles @ W1p (xT in 2 chunks) ----
            u1_t = upool.tile([P, WPC * HID], bf, tag="u1")
            HWPC = WPC // 2
            for half in range(2):
                xc = stpool.tile([P, HWPC * P], bf, tag="stg64",
                                 name=f"xc{half}")
                nc.sync.dma_start(out=xc[:],
                                  in_=xT_d[:, half * HWPC * P:
                                           (half + 1) * HWPC * P])
                for b in range((HWPC + 7) // 8):
                    pm = pmm.tile([P, 512], f32, space="PSUM", tag="pm")
                    ts = range(b * 8, min((b + 1) * 8, HWPC))
                    for i, t in enumerate(ts):
                        nc.tensor.matmul(out=pm[:, i * HID:(i + 1) * HID],
                                         lhsT=xc[:, t * P:(t + 1) * P],
                                         rhs=w1_t[:], start=True, stop=True)
                    nts = len(ts)
                    t0 = half * HWPC + b * 8
                    nc.vector.tensor_copy(
                        out=u1_t[:, t0 * HID:(t0 + nts) * HID],
                        in_=pm[:, 0:nts * HID])

            # table1 rows ell = p*WPC+w (strided 256B)
            nc.sync.dma_start(
                out=bassm.AP(tensor=tab1_d[:].tensor, offset=0,
                             ap=[[WPC * TBW, P], [TBW, WPC], [1, HID]]),
                in_=u1_t[:].rearrange("p (w f) -> p w f", f=HID),
            )

            # ---- per-edge stream: gather + one-hot matmul aggregation ----
            HWPC = WPC // 2                  # windows per (core, half)
            HWIN = NWIN // 2                 # global windows per half
            HROWS = NSLOT // 2               # acc rows per half

            def edge_stream(tab, acc, rs, fw, nwg, layer):
                """Gather dest-sorted messages, build one-hot S tiles on DVE,
                accumulate per-window sums in PSUM on the PE, stage each
                (half, core) chunk in SBUF (Act copy), write it contiguously,
                and fire the half's ReduceScatter once its last chunk is
                written (delayed by one bin so Pool desc-gen isn't stalled
                on the staging write)."""
                cur_pm = [None]
                cur_stg = [None]
                pending_rs = []

                def emit_rs(h):
                    nc.gpsimd.collective_compute(
                        "ReduceScatter", mybir.AluOpType.add,
                        replica_groups=[list(range(NCORES))],
                        ins=[bassm.AP(tensor=acc[:].tensor,
                                      offset=h * HROWS * fw,
                                      ap=[[fw, HROWS], [1, fw]])],
                        outs=[bassm.AP(tensor=rs[:].tensor,
                                       offset=h * (HROWS // NCORES) * fw,
                                       ap=[[fw, HROWS // NCORES], [1, fw]])],
                    )

                for (t_lo, nt, w_lo, nw) in bins:
                    gv = gpool.tile([P, NTBIN * HID], bf, tag="gv",
                                    name=f"gv{layer}_{t_lo}")
                    _dma_gather_raw(
                        nc.gpsimd, nc,
                        gv[:].rearrange("p (t f) -> p t f", f=fw)[:, 0:nt, :],
                        bassm.AP(tensor=tab[:].tensor, offset=0,
                                 ap=[[TBW, SPC], [1, fw]]),
                        gidx_t[:, t_lo * 8:(t_lo + nt) * 8], nt * P, fw, TBW,
                        single_packet=False, queue_num=0)
                    while pending_rs:
                        emit_rs(pending_rs.pop())
                    st = spool.tile([P, NTBIN * P], bf, tag="s",
                                    name=f"s{layer}_{t_lo}")
                    for trel in range(nt):
                        nc.vector.tensor_scalar(
                            out=st[:, trel * P:(trel + 1) * P],
                            in0=iota_t[:],
                            scalar1=dlocf_t[:, t_lo + trel:t_lo + trel + 1],
                            scalar2=None, op0=mybir.AluOpType.is_equal)
                    for W in range(w_lo, w_lo + nw):
                        h, c_, wi = W // HWIN, (W % HWIN) // HWPC, W % HWPC
                        g0 = (wi // nwg) * nwg
                        g1_ = min(g0 + nwg, HWPC)
                        if wi == 0:
                            cur_stg[0] = stpool.tile([P, HWPC * fw], bf,
                                                     tag="stg64",
                                                     name=f"stg{layer}_{h}_{c_}")
                        if wi == g0:
                            cur_pm[0] = pagg.tile([P, 512], f32, space="PSUM",
                                                  tag="agg",
                                                  name=f"agg{layer}_{W}")
                        wrel = wi - g0
                        for k in range(T[W]):
                            trel = win_tile0[W] - t_lo + k
                            nc.tensor.matmul(
                                out=cur_pm[0][:, wrel * fw:(wrel + 1) * fw],
                                lhsT=st[:, trel * P:(trel + 1) * P],
                                rhs=gv[:, trel * fw:(trel + 1) * fw],
                                start=(k == 0), stop=(k == T[W] - 1))
                        if wi == g1_ - 1:
                            nwv = g1_ - g0
                            nc.scalar.activation(
                                out=cur_stg[0][:, g0 * fw:g1_ * fw],
                                in_=cur_pm[0][:, 0:nwv * fw],
                                func=mybir.ActivationFunctionType.Copy)
                        if wi == HWPC - 1:
                            nc.sync.dma_start(
                                out=bassm.AP(tensor=acc[:].tensor,
                                             offset=(h * HROWS
                                                     + c_ * HROWS // NCORES) * fw,
                                             ap=[[HWPC * fw, P],
                                                 [1, HWPC * fw]]),
                                in_=cur_stg[0][:])
                            if c_ == NCORES - 1:
                                pending_rs.append(h)
                while pending_rs:
                    emit_rs(pending_rs.pop())

            edge_stream(tab1_d, acc1_d, rs1_d, HID, 8, 1)

            # ---- post1 per half: z = relu(dinv*agg + dinv*u1 + T1) ----
            def precompute_self(u, dvt, tt, fw, tag):
                pre = wpool.tile([P, WPC, fw], bf, tag=f"pre{tag}")
                u3 = u[:].rearrange("p (w f) -> p w f", f=fw)
                nc.vector.tensor_tensor(
                    out=pre[:], in0=u3,
                    in1=dvt[:, :, None].to_broadcast([P, WPC, fw]),
                    op=mybir.AluOpType.mult)
                nc.vector.tensor_tensor(
                    out=pre[:], in0=pre[:],
                    in1=tt[:, None, :].to_broadcast([P, WPC, fw]),
                    op=mybir.AluOpType.add)
                return pre

            def read_agg(dst, rs, fw, h):
                # half h's RS output rows r = p*HWPC + w' -> dst[:, h*HWPC+w', :]
                nc.sync.dma_start(
                    out=dst[:, h * HWPC * fw:(h + 1) * HWPC * fw],
                    in_=bassm.AP(tensor=rs[:].tensor,
                                 offset=h * (HROWS // NCORES) * fw,
                                 ap=[[HWPC * fw, P], [1, HWPC * fw]]),
                )

            def post(agg, pre, dvt, fw, out_t, h):
                lo, nt = h * HWPC, HWPC
                a3 = agg[:].rearrange("p (w f) -> p w f", f=fw)
                tmp = wpool.tile([P, nt, fw], bf, tag=f"pa{fw}",
                                 name=f"pa{fw}_{h}")
                nc.vector.tensor_tensor(
                    out=tmp[:], in0=a3[:, lo:lo + nt, :],
                    in1=dvt[:, lo:lo + nt, None].to_broadcast([P, nt, fw]),
                    op=mybir.AluOpType.mult)
                nc.vector.tensor_tensor(out=tmp[:], in0=tmp[:],
                                        in1=pre[:, lo:lo + nt, :],
                                        op=mybir.AluOpType.add)
                nc.scalar.activation(
                    out=out_t[:, lo * fw:(lo + nt) * fw],
                    in_=tmp[:].rearrange("p t f -> p (t f)"),
                    func=mybir.ActivationFunctionType.Relu)

            # u1 is pre-scaled by dinv, so its self-term multiplier is dinv.
            pre1 = precompute_self(u1_t, dinv_t, t1_t, HID, "1")
            agg1 = upool.tile([P, WPC * HID], bf, tag="agg1")
            z_t = upool.tile([P, WPC * HID], bf, tag="z")
            u2_t = upool.tile([P, WPC * HID2], bf, tag="u2")
            t2v = wpool.tile([P, WPC, HID2], bf, tag="t2v")

            def dense2_half(h):
                # u2 = z @ W2p and tab2 = dinv*u2 for half h's window columns
                wlist = list(range(h * HWPC, (h + 1) * HWPC))
                for b in range((len(wlist) + 15) // 16):
                    pm = pmm.tile([P, 512], f32, space="PSUM", tag="pm",
                                  name=f"pm2_{h}_{b}")
                    ts = wlist[b * 16:(b + 1) * 16]
                    for s4 in range(0, len(ts), 4):
                        sub = ts[s4:s4 + 4]
                        tr = ptr.tile([HID, 512], bf, space="PSUM", tag="tr")
                        for i, t in enumerate(sub):
                            nc.tensor.transpose(
                                out=tr[:, i * P:(i + 1) * P],
                                in_=z_t[:, t * HID:(t + 1) * HID],
                                identity=ident[:])
                        zc = zcpool.tile([HID, 512], bf, tag="zc",
                                         name=f"zc{h}_{b}_{s4}")
                        nc.vector.tensor_copy(out=zc[:, 0:len(sub) * P],
                                              in_=tr[:, 0:len(sub) * P])
                        for i, t in enumerate(sub):
                            nc.tensor.matmul(
                                out=pm[:, (s4 + i) * HID2:(s4 + i + 1) * HID2],
                                lhsT=zc[:, i * P:(i + 1) * P],
                                rhs=w2_t[:], start=True, stop=True)
                    nts = len(ts)
                    b0 = ts[0]
                    nc.vector.tensor_copy(
                        out=u2_t[:, b0 * HID2:(b0 + nts) * HID2],
                        in_=pm[:, 0:nts * HID2])
                    nc.vector.tensor_tensor(
                        out=t2v[:, b0:b0 + nts, :],
                        in0=pm[:, 0:nts * HID2].rearrange("p (t f) -> p t f",
                                                          f=HID2),
                        in1=dinv_t[:, b0:b0 + nts, None]
                            .to_broadcast([P, nts, HID2]),
                        op=mybir.AluOpType.mult)
                nc.sync.dma_start(
                    out=bassm.AP(tensor=tab2_d[:].tensor,
                                 offset=h * HWPC * TBW,
                                 ap=[[WPC * TBW, P], [TBW, HWPC], [1, HID2]]),
                    in_=t2v[:, h * HWPC:(h + 1) * HWPC, :])

            for h in range(2):
                read_agg(agg1, rs1_d, HID, h)
                post(agg1, pre1, dinv_t, HID, z_t, h)
                dense2_half(h)

            edge_stream(tab2_d, acc2_d, rs2_d, HID2, 16, 2)

            # ---- post2 per half + fc ----
            # table2 values dinv*u2 give self term dinv2*u2; u2 is unscaled.
            pre2 = precompute_self(u2_t, dinv2_t, t2_t, HID2, "2")
            agg2 = upool.tile([P, WPC * HID2], bf, tag="agg2")
            h2_t = upool.tile([P, WPC * HID2], bf, tag="h2")
            out_t = upool.tile([P, WPC], f32, tag="out")
            for h in range(2):
                read_agg(agg2, rs2_d, HID2, h)
                post(agg2, pre2, dinv_t, HID2, h2_t, h)
                prod = wpool.tile([P, HWPC, HID2], bf, tag="prod",
                                  name=f"prod_{h}")
                nc.vector.tensor_tensor(
                    out=prod[:],
                    in0=h2_t[:, h * HWPC * HID2:(h + 1) * HWPC * HID2]
                        .rearrange("p (w f) -> p w f", f=HID2),
                    in1=fcw_t[:, None, :].to_broadcast([P, HWPC, HID2]),
                    op=mybir.AluOpType.mult)
                nc.vector.reduce_sum(
                    out=out_t[:, h * HWPC:(h + 1) * HWPC, None], in_=prod[:],
                    axis=mybir.AxisListType.X)
            nc.sync.dma_start(out=y_d[:], in_=out_t[:])

    nc.compile()
    return nc


# ----------------------------------------------------------------------
# entry points
# ----------------------------------------------------------------------
def prepare(inputs):
    inputs = {k: np.asarray(v) for k, v in inputs.items()}
    in_maps, consts = host_prep(**inputs)
    nc = build_bass(consts["T"], consts["win_tile0"], consts["bins"],
                    consts["ntiles"])
    return nc, in_maps, consts


def execute(nc, in_maps):
    from concourse.bass_utils import run_bass_kernel_spmd
    return run_bass_kernel_spmd(nc, in_maps, core_ids=list(range(NCORES)))


def unshard(res, consts):
    y = np.zeros((N_NODES, 1), np.float32)
    fcb = consts["fcb"]
    nos = consts["node_of_slot"]
    for c in range(NCORES):
        v = np.asarray(res.results[c]["y"], np.float32).reshape(-1)  # ell order
        valid = nos[c] >= 0
        y[nos[c][valid], 0] = v[valid] + fcb
    return y


def kernel(**inputs):
    nc, in_maps, consts = prepare(inputs)
    res = execute(nc, in_maps)
    return unshard(res, consts)


# revision 23
# speedup vs baseline: 1.5941x; 1.0488x over previous
"""Distributed 2-layer GCN (BangaloreGCN) on 8 Trainium2 NeuronCores.

Matmul-aggregation design (v3):
  * Source-partitioned: core c owns nodes [c*6250, (c+1)*6250) and the
    edges whose SOURCE it owns.  Per layer, each core computes a local
    message table (dinv-scaled dense transform of its own nodes), then
    gathers per-edge messages with dma_gather in DEST-SORTED order.
  * The scatter side is done on the PE array instead of dma_scatter_add:
    the global dest space is split into 400 windows of 128 slots.  Each
    128-edge gathered tile (edge i -> partition i%128) is multiplied by
    a one-hot "selection" matrix S [128 edges, 128 dests] built on the
    DVE (is_equal of per-edge dest-column vs an iota row), accumulating
    partial sums for a window directly in PSUM.  This removes the
    scatter DMA, the accumulator zeroing, and the scatter descriptor
    generation of v2 entirely.
  * Node -> slot assignment is chosen by a greedy packer so that every
    window needs at most 2 tiles per source core (max in-window edge
    count <= 256 for all 8 cores): the SPMD-static stream is ~103k
    indices per core vs 800k/8 = 100k real edges.
  * Slot labeling ell = p*50 + w makes the per-core accumulator chunk
    contiguous per partition, so the PSUM->DRAM staging writes run at
    full DMA rate, and the ReduceScatter chunk c is exactly core c's
    own slots.  Both layers share the identical edge stream, gather
    indices, and S structure (S is rebuilt per layer; it does not fit
    in SBUF).
"""

import sys

sys.path.insert(0, "/opt/trn_rl_repo")

import ml_dtypes
import numpy as np

F16 = np.float16

# ---- problem constants ----
N_NODES = 50000
IN_CH = 128
HID = 64
HID2 = 32
BN_EPS = 1e-5

NCORES = 8
P = 128
WPC = 50                   # windows (tiles) per core
SPC = P * WPC              # 6400 slots per core
NSLOT = NCORES * SPC       # 51200
NWIN = NCORES * WPC        # 400 global windows
REAL = N_NODES // NCORES   # 6250 real nodes per core
WCAP = 127                 # real nodes per window (p=127 spare everywhere)
TBW = 128                  # table row width in bf16 elems (256B stride)
SPARE_ROW = WCAP * WPC     # a slot that is spare on every core (p=127,w=0)
NTBIN = int(__import__("os").environ.get("KNTBIN", "48"))  # tiles per gather bin


# ----------------------------------------------------------------------
# host-side preparation
# ----------------------------------------------------------------------
def _wrap_idx(arr):
    """[n] int -> [128, n/16] int16 image (16-partition wrap, replicated)."""
    ni = arr.shape[0]
    assert ni % 16 == 0
    blk = arr.reshape(ni // 16, 16).T.astype(np.int16)
    return np.tile(blk, (8, 1))


def _pack_windows(Mi):
    """Greedy: assign nodes (rows of Mi [REAL, 8] = per-source-core indeg)
    to WPC windows, minimizing the max per-core in-window load, capped at
    WCAP nodes per window.  Returns win[i] for nodes in degree-sorted
    order and that order."""
    srt = np.argsort(-Mi.sum(1), kind="stable")
    Ms = Mi[srt]
    loads = np.zeros((WPC, NCORES), np.int64)
    cnt = np.zeros(WPC, np.int64)
    win = np.empty(REAL, np.int64)
    big = 1 << 40
    for i in range(REAL):
        cand = (loads + Ms[i]).max(1) + (cnt >= WCAP) * big
        w = int(np.argmin(cand))
        win[i] = w
        loads[w] += Ms[i]
        cnt[w] += 1
    return srt, win


def host_prep(x, edge_index, W1, b1, W2, b2, fcW, fcb,
              g1, be1, rm1, rv1, g2, be2, rm2, rv2):
    row = np.asarray(edge_index[0], np.int64)
    col = np.asarray(edge_index[1], np.int64)
    x = np.asarray(x, np.float32)

    deg = np.bincount(col, minlength=N_NODES).astype(np.float32) + 1.0
    dinv = (1.0 / np.sqrt(deg)).astype(np.float32)

    owner_src = row // REAL

    # ---- node -> slot assignment (window packing per dest core) ----
    M = np.zeros((N_NODES, NCORES), np.int32)
    np.add.at(M, (col, owner_src), 1)
    slot_of_node = np.full(N_NODES, -1, np.int64)      # global slot
    node_of_slot = np.full((NCORES, SPC), -1, np.int64)
    for c in range(NCORES):
        nodes = np.arange(c * REAL, (c + 1) * REAL)
        srt, win = _pack_windows(M[nodes])
        # p = rank within window (stable in assignment order)
        o2 = np.argsort(win, kind="stable")
        wsort = win[o2]
        first = np.zeros(REAL, np.int64)
        starts = np.r_[0, np.flatnonzero(np.diff(wsort)) + 1]
        first[starts] = starts
        first = np.maximum.accumulate(first)
        p_of = np.empty(REAL, np.int64)
        p_of[o2] = np.arange(REAL) - first
        assert p_of.max() < WCAP
        ell = p_of * WPC + win
        slot_of_node[nodes[srt]] = c * SPC + ell
        node_of_slot[c, ell] = nodes[srt]

    # ---- per-edge window/column/source-row ----
    # Global window order is HALF-major (h = win // HWPC), then dest core,
    # then window-within-half: the ReduceScatter for half h covers acc rows
    # [h*NSLOT/2, (h+1)*NSLOT/2) and fires as soon as half h's windows are
    # done -- half A's RS overlaps half B's edge stream.
    gdst = slot_of_node[col]
    c_d, ell_d = gdst // SPC, gdst % SPC
    p_d, w_d = ell_d // WPC, ell_d % WPC
    HWPC = WPC // 2
    gwin = (w_d // HWPC) * (NCORES * HWPC) + c_d * HWPC + (w_d % HWPC)
    src_slot = slot_of_node[row] % SPC                 # local table row

    # ---- per-window tile counts (static, max over source cores) ----
    cnts = np.zeros((NWIN, NCORES), np.int64)
    np.add.at(cnts, (gwin, owner_src), 1)
    T = np.maximum(1, -(-cnts.max(1) // P)).astype(np.int64)   # [NWIN]
    win_tile0 = np.r_[0, np.cumsum(T)][:-1]
    ntiles = int(T.sum())
    stream = ntiles * P

    # ---- per-core gather index + dest-column streams ----
    gidx_s = np.full((NCORES, stream), SPARE_ROW, np.int64)
    dloc_s = np.full((NCORES, stream), 255, np.int64)
    for h in range(NCORES):
        sel = owner_src == h
        gw, ss, pd = gwin[sel], src_slot[sel], p_d[sel]
        o = np.argsort(gw, kind="stable")
        gw, ss, pd = gw[o], ss[o], pd[o]
        starts = np.r_[0, np.flatnonzero(np.diff(gw)) + 1]
        first = np.zeros(len(gw), np.int64)
        first[starts] = starts
        first = np.maximum.accumulate(first)
        rank = np.arange(len(gw)) - first
        pos = win_tile0[gw] * P + rank
        gidx_s[h, pos] = ss
        dloc_s[h, pos] = pd

    # ---- gather bins: consecutive whole windows, <= NTBIN tiles;
    #      forced break at the half boundary ----
    bins = []                                          # (t_lo, nt, w_lo, nw)
    w_lo, t_lo = 0, 0
    for W in range(NWIN):
        if W > w_lo and ((win_tile0[W] + T[W] - t_lo) > NTBIN
                         or W == NWIN // 2):
            bins.append((t_lo, int(win_tile0[W] - t_lo), w_lo, W - w_lo))
            w_lo, t_lo = W, int(win_tile0[W])
    bins.append((t_lo, ntiles - t_lo, w_lo, NWIN - w_lo))
    assert max(b[1] for b in bins) <= NTBIN

    # ---- BN folding ----
    S1c = (np.asarray(g1) / np.sqrt(np.asarray(rv1) + BN_EPS)).astype(np.float32)
    T1 = ((np.asarray(b1) - np.asarray(rm1)) * S1c + np.asarray(be1)).astype(np.float32)
    S2c = (np.asarray(g2) / np.sqrt(np.asarray(rv2) + BN_EPS)).astype(np.float32)
    T2 = ((np.asarray(b2) - np.asarray(rm2)) * S2c + np.asarray(be2)).astype(np.float32)
    W1p = (np.asarray(W1) * S1c[None, :]).astype(np.float32)
    W2p = (np.asarray(W2) * S2c[None, :]).astype(np.float32)

    # ---- per-core tensors ----
    # xT column j holds slot (j%128)*WPC + j//128 so dense tile t yields
    # u1[p, t*HID:..] = slot p*WPC + t.
    colperm = (np.arange(SPC) % P) * WPC + (np.arange(SPC) // P)
    iota = np.tile(np.arange(P, dtype=np.float32)[None, :], (P, 1))
    in_maps = []
    for c in range(NCORES):
        xs = np.zeros((SPC, IN_CH), np.float32)
        dv = np.zeros(SPC, np.float32)
        valid = node_of_slot[c] >= 0
        nd = node_of_slot[c][valid]
        xs[valid] = x[nd] * dinv[nd, None]
        dv[valid] = dinv[nd]
        xs = xs[colperm]                                # [SPC(col j), IN_CH]
        dv_im = dv.reshape(P, WPC)                      # [p, w]
        in_maps.append({
            "xT": np.ascontiguousarray(xs.T).astype(F16),
            "gidx": _wrap_idx(gidx_s[c]),
            "dloc": np.ascontiguousarray(
                dloc_s[c].reshape(ntiles, P).T).astype(F16),
            "dinv": dv_im.astype(F16),
            "iota": iota.astype(F16),
            "w1": W1p.astype(F16),
            "w2": W2p.astype(F16),
            "t1": np.tile(T1[None, :], (P, 1)).astype(F16),
            "t2": np.tile(T2[None, :], (P, 1)).astype(F16),
            "fcw": np.tile(np.asarray(fcW, np.float32).reshape(1, -1),
                           (P, 1)).astype(F16),
        })

    consts = dict(T=T.tolist(), win_tile0=win_tile0.tolist(), bins=bins,
                  ntiles=ntiles, node_of_slot=node_of_slot,
                  fcb=float(np.asarray(fcb).reshape(-1)[0]))
    return in_maps, consts


# ----------------------------------------------------------------------
# raw dma_gather (elem_size below 256B; stride multiple of 256B)
# ----------------------------------------------------------------------
def _dma_gather_raw(gp, bassmod, out_ap, in_ap, idxs_ap, num_idxs, elem_size,
                    elem_step, single_packet=True, queue_num=0):
    import concourse.mybir as mybir
    from concourse import ap_utils
    from concourse.bass import MemorySpace, exact_div, round_up_to_multiple

    assert idxs_ap.dtype == mybir.dt.int16
    assert in_ap.dtype == out_ap.dtype
    assert in_ap.space == MemorySpace.DRAM
    assert idxs_ap.space == MemorySpace.SBUF and out_ap.space == MemorySpace.SBUF
    assert ap_utils.ap_is_contiguous(out_ap.ap[1:])
    assert ap_utils.ap_is_contiguous(idxs_ap.ap[1:])
    assert in_ap.ap[-1][1] == out_ap.ap[-1][1] == elem_size
    assert out_ap.ap[0][1] * out_ap.ap[1][1] == round_up_to_multiple(num_idxs, 128)
    assert in_ap.ap[0][0] == elem_step
    stride_bytes_256 = exact_div(elem_step * mybir.dt.size(in_ap.dtype), 256)
    assert stride_bytes_256 < 256
    return gp.add_instruction(
        mybir.InstDMAGatherAnt(
            name=bassmod.get_next_instruction_name(),
            ins=[*gp.lower_ap_dma(in_ap, for_custom_bir_dma=True),
                 gp.lower_ap(idxs_ap),
                 gp.lower_val_access(gp.to_reg(num_idxs))],
            outs=[gp.lower_ap(out_ap)],
            transpose=False,
            num_idxs=num_idxs,
            elem_size=elem_size,
            stride_bytes_256=stride_bytes_256,
            gen_mode=0,
            single_packet=single_packet,
            queue_num=queue_num,
            sbuf_tokens_per_rank=0,
            sbuf_free_dim_per_rank=0,
            sbuf_free_dim_pad_per_rank=0,
            sbuf_byte_offset=0,
        ))


# ----------------------------------------------------------------------
# device program
# ----------------------------------------------------------------------
def build_bass(T, win_tile0, bins, ntiles):
    import concourse.bacc as bacc
    import concourse.bass as bassm
    import concourse.mybir as mybir
    import concourse.tile as tile
    from concourse.masks import make_identity

    f32 = mybir.dt.float32
    bf = mybir.dt.float16
    i16 = mybir.dt.int16

    import os as _os
    nc = bacc.Bacc("TRN2", target_bir_lowering=False,
                   dynamic_dma_scratch_size=int(_os.environ.get("KSCRATCH", "49152")),
                   num_swdge_queues=1)
    xT_d = nc.dram_tensor("xT", [P, SPC], bf, kind="ExternalInput")
    gidx_d = nc.dram_tensor("gidx", [P, ntiles * 8], i16, kind="ExternalInput")
    dloc_d = nc.dram_tensor("dloc", [P, ntiles], bf, kind="ExternalInput")
    dinv_d = nc.dram_tensor("dinv", [P, WPC], bf, kind="ExternalInput")
    iota_d = nc.dram_tensor("iota", [P, P], bf, kind="ExternalInput")
    w1_d = nc.dram_tensor("w1", [IN_CH, HID], bf, kind="ExternalInput")
    w2_d = nc.dram_tensor("w2", [HID, HID2], bf, kind="ExternalInput")
    t1_d = nc.dram_tensor("t1", [P, HID], bf, kind="ExternalInput")
    t2_d = nc.dram_tensor("t2", [P, HID2], bf, kind="ExternalInput")
    fcw_d = nc.dram_tensor("fcw", [P, HID2], bf, kind="ExternalInput")
    y_d = nc.dram_tensor("y", [P, WPC], f32, kind="ExternalOutput")

    with tile.TileContext(nc) as tc:
        with (
            tc.tile_pool(name="const", bufs=1) as cpool,
            tc.tile_pool(name="work", bufs=1) as upool,
            tc.tile_pool(name="g", bufs=int(_os.environ.get("KGBUF", "3"))) as gpool,
            tc.tile_pool(name="sel", bufs=int(_os.environ.get("KSBUF", "3"))) as spool,
            tc.tile_pool(name="stage", bufs=2) as stpool,
            tc.tile_pool(name="zc", bufs=2) as zcpool,
            tc.tile_pool(name="tmp", bufs=1) as wpool,
            tc.tile_pool(name="pmm", bufs=2, space="PSUM") as pmm,
            tc.tile_pool(name="pagg", bufs=3, space="PSUM") as pagg,
            tc.tile_pool(name="ptr", bufs=2, space="PSUM") as ptr,
            tc.tile_pool(name="dram", bufs=1, space="DRAM") as dpool,
        ):
            # ---- DRAM scratch ----
            tab1_d = dpool.tile([SPC, TBW], bf)
            tab2_d = dpool.tile([SPC, TBW], bf)
            acc1_d = dpool.tile([NSLOT, HID], bf)
            acc2_d = dpool.tile([NSLOT, HID2], bf)
            rs1_d = dpool.tile([SPC, HID], bf)
            rs2_d = dpool.tile([SPC, HID2], bf)

            # ---- constants ----
            gidx_t = cpool.tile([P, ntiles * 8], i16)
            nc.scalar.dma_start(out=gidx_t[:], in_=gidx_d[:])
            dloc_t = cpool.tile([P, ntiles], bf)
            nc.scalar.dma_start(out=dloc_t[:], in_=dloc_d[:])
            dinv_t = cpool.tile([P, WPC], bf)
            nc.scalar.dma_start(out=dinv_t[:], in_=dinv_d[:])
            iota_t = cpool.tile([P, P], bf)
            nc.scalar.dma_start(out=iota_t[:], in_=iota_d[:])
            w1_t = cpool.tile([IN_CH, HID], bf)
            nc.scalar.dma_start(out=w1_t[:], in_=w1_d[:])
            w2_t = cpool.tile([HID, HID2], bf)
            nc.scalar.dma_start(out=w2_t[:], in_=w2_d[:])
            t1_t = cpool.tile([P, HID], bf)
            nc.scalar.dma_start(out=t1_t[:], in_=t1_d[:])
            t2_t = cpool.tile([P, HID2], bf)
            nc.scalar.dma_start(out=t2_t[:], in_=t2_d[:])
            fcw_t = cpool.tile([P, HID2], bf)
            nc.scalar.dma_start(out=fcw_t[:], in_=fcw_d[:])
            ident = cpool.tile([P, P], bf)
            make_identity(nc, ident[:])

            dinv2_t = cpool.tile([P, WPC], bf)
            nc.vector.tensor_tensor(out=dinv2_t[:], in0=dinv_t[:], in1=dinv_t[:],
                                    op=mybir.AluOpType.mult)
            # f32 copy of dloc (tensor_scalar is_equal wants an f32 scalar AP)
            dlocf_t = cpool.tile([P, ntiles], f32)
            nc.vector.tensor_copy(out=dlocf_t[:], in_=dloc_t[:])

            # ---- L1 dense: u1 = (dinv*x)^T tiles @ W1p (xT in 2 chunks) ----
            u1_t = upool.tile([P, WPC * HID], bf, tag="u1")
            HWPC = WPC // 2
            for half in range(2):
                xc = stpool.tile([P, HWPC * P], bf, tag="stg64",
                                 name=f"xc{half}")
                nc.sync.dma_start(out=xc[:],
                                  in_=xT_d[:, half * HWPC * P:
                                           (half + 1) * HWPC * P])
                for b in range((HWPC + 7) // 8):
                    pm = pmm.tile([P, 512], f32, space="PSUM", tag="pm")
                    ts = range(b * 8, min((b + 1) * 8, HWPC))
                    for i, t in enumerate(ts):
                        nc.tensor.matmul(out=pm[:, i * HID:(i + 1) * HID],
                                         lhsT=xc[:, t * P:(t + 1) * P],
                                         rhs=w1_t[:], start=True, stop=True)
                    nts = len(ts)
                    t0 = half * HWPC + b * 8
                    nc.scalar.activation(
                        out=u1_t[:, t0 * HID:(t0 + nts) * HID],
                        in_=pm[:, 0:nts * HID],
                        func=mybir.ActivationFunctionType.Copy)
                    # table1 rows ell = p*WPC+w (strided 256B), sliced so the
                    # write overlaps the remaining dense groups
                    nc.sync.dma_start(
                        out=bassm.AP(tensor=tab1_d[:].tensor, offset=t0 * TBW,
                                     ap=[[WPC * TBW, P], [TBW, nts], [1, HID]]),
                        in_=u1_t[:, t0 * HID:(t0 + nts) * HID]
                            .rearrange("p (w f) -> p w f", f=HID),
                    )

            # ---- per-edge stream: gather + one-hot matmul aggregation ----
            HWPC = WPC // 2                  # windows per (core, half)
            HWIN = NWIN // 2                 # global windows per half
            HROWS = NSLOT // 2               # acc rows per half

            def build_s(bi, layer):
                t_lo, nt = bins[bi][0], bins[bi][1]
                st = spool.tile([P, NTBIN * P], bf, tag="s",
                                name=f"s{layer}_{t_lo}")
                for trel in range(nt):
                    nc.vector.tensor_scalar(
                        out=st[:, trel * P:(trel + 1) * P],
                        in0=iota_t[:],
                        scalar1=dlocf_t[:, t_lo + trel:t_lo + trel + 1],
                        scalar2=None, op0=mybir.AluOpType.is_equal)
                return st

            def edge_stream(tab, acc, rs, fw, nwg, layer, prebuilt=()):
                """Gather dest-sorted messages, build one-hot S tiles on DVE,
                accumulate per-window sums in PSUM on the PE, stage each
                (half, core) chunk in SBUF (Act copy), write it contiguously,
                and fire the half's ReduceScatter once its last chunk is
                written (delayed by one bin so Pool desc-gen isn't stalled
                on the staging write)."""
                cur_pm = [None]
                cur_stg = [None]
                pending_rs = []

                def emit_rs(h):
                    nc.gpsimd.collective_compute(
                        "ReduceScatter", mybir.AluOpType.add,
                        replica_groups=[list(range(NCORES))],
                        ins=[bassm.AP(tensor=acc[:].tensor,
                                      offset=h * HROWS * fw,
                                      ap=[[fw, HROWS], [1, fw]])],
                        outs=[bassm.AP(tensor=rs[:].tensor,
                                       offset=h * (HROWS // NCORES) * fw,
                                       ap=[[fw, HROWS // NCORES], [1, fw]])],
                    )

                for bi, (t_lo, nt, w_lo, nw) in enumerate(bins):
                    gv = gpool.tile([P, NTBIN * HID], bf, tag="gv",
                                    name=f"gv{layer}_{t_lo}")
                    _dma_gather_raw(
                        nc.gpsimd, nc,
                        gv[:].rearrange("p (t f) -> p t f", f=fw)[:, 0:nt, :],
                        bassm.AP(tensor=tab[:].tensor, offset=0,
                                 ap=[[TBW, SPC], [1, fw]]),
                        gidx_t[:, t_lo * 8:(t_lo + nt) * 8], nt * P, fw, TBW,
                        single_packet=False, queue_num=0)
                    while pending_rs:
                        emit_rs(pending_rs.pop())
                    st = prebuilt[bi] if bi < len(prebuilt) else build_s(bi, layer)
                    for W in range(w_lo, w_lo + nw):
                        h, c_, wi = W // HWIN, (W % HWIN) // HWPC, W % HWPC
                        g0 = (wi // nwg) * nwg
                        g1_ = min(g0 + nwg, HWPC)
                        if wi == 0:
                            cur_stg[0] = stpool.tile([P, HWPC * fw], bf,
                                                     tag="stg64",
                                                     name=f"stg{layer}_{h}_{c_}")
                        if wi == g0:
                            cur_pm[0] = pagg.tile([P, 512], f32, space="PSUM",
                                                  tag="agg",
                                                  name=f"agg{layer}_{W}")
                        wrel = wi - g0
                        for k in range(T[W]):
                            trel = win_tile0[W] - t_lo + k
                            nc.tensor.matmul(
                                out=cur_pm[0][:, wrel * fw:(wrel + 1) * fw],
                                lhsT=st[:, trel * P:(trel + 1) * P],
                                rhs=gv[:, trel * fw:(trel + 1) * fw],
                                start=(k == 0), stop=(k == T[W] - 1))
                        if wi == g1_ - 1:
                            nwv = g1_ - g0
                            nc.scalar.activation(
                                out=cur_stg[0][:, g0 * fw:g1_ * fw],
                                in_=cur_pm[0][:, 0:nwv * fw],
                                func=mybir.ActivationFunctionType.Copy)
                        if wi == HWPC - 1:
                            nc.sync.dma_start(
                                out=bassm.AP(tensor=acc[:].tensor,
                                             offset=(h * HROWS
                                                     + c_ * HROWS // NCORES) * fw,
                                             ap=[[HWPC * fw, P],
                                                 [1, HWPC * fw]]),
                                in_=cur_stg[0][:])
                            if c_ == NCORES - 1:
                                pending_rs.append(h)
                while pending_rs:
                    emit_rs(pending_rs.pop())

            edge_stream(tab1_d, acc1_d, rs1_d, HID, 8, 1)

            # ---- post1 per half: z = relu(dinv*agg + dinv*u1 + T1) ----
            def precompute_self(u, dvt, tt, fw, tag):
                pre = wpool.tile([P, WPC, fw], bf, tag=f"pre{tag}")
                u3 = u[:].rearrange("p (w f) -> p w f", f=fw)
                nc.vector.tensor_tensor(
                    out=pre[:], in0=u3,
                    in1=dvt[:, :, None].to_broadcast([P, WPC, fw]),
                    op=mybir.AluOpType.mult)
                nc.vector.tensor_tensor(
                    out=pre[:], in0=pre[:],
                    in1=tt[:, None, :].to_broadcast([P, WPC, fw]),
                    op=mybir.AluOpType.add)
                return pre

            def read_agg(dst, rs, fw, h):
                # half h's RS output rows r = p*HWPC + w' -> dst[:, h*HWPC+w', :]
                nc.sync.dma_start(
                    out=dst[:, h * HWPC * fw:(h + 1) * HWPC * fw],
                    in_=bassm.AP(tensor=rs[:].tensor,
                                 offset=h * (HROWS // NCORES) * fw,
                                 ap=[[HWPC * fw, P], [1, HWPC * fw]]),
                )

            def post(agg, pre, dvt, fw, out_t, h):
                lo, nt = h * HWPC, HWPC
                a3 = agg[:].rearrange("p (w f) -> p w f", f=fw)
                tmp = wpool.tile([P, nt, fw], bf, tag=f"pa{fw}",
                                 name=f"pa{fw}_{h}")
                nc.vector.tensor_tensor(
                    out=tmp[:], in0=a3[:, lo:lo + nt, :],
                    in1=dvt[:, lo:lo + nt, None].to_broadcast([P, nt, fw]),
                    op=mybir.AluOpType.mult)
                nc.vector.tensor_tensor(out=tmp[:], in0=tmp[:],
                                        in1=pre[:, lo:lo + nt, :],
                                        op=mybir.AluOpType.add)
                nc.scalar.activation(
                    out=out_t[:, lo * fw:(lo + nt) * fw],
                    in_=tmp[:].rearrange("p t f -> p (t f)"),
                    func=mybir.ActivationFunctionType.Relu)

            # u1 is pre-scaled by dinv, so its self-term multiplier is dinv.
            pre1 = precompute_self(u1_t, dinv_t, t1_t, HID, "1")
            agg1 = upool.tile([P, WPC * HID], bf, tag="agg1")
            z_t = upool.tile([P, WPC * HID], bf, tag="z")
            u2_t = upool.tile([P, WPC * HID2], bf, tag="u2")
            t2v = wpool.tile([P, WPC, HID2], bf, tag="t2v")

            def dense2_half(h):
                # u2 = z @ W2p and tab2 = dinv*u2 for half h's window columns
                wlist = list(range(h * HWPC, (h + 1) * HWPC))
                for b in range((len(wlist) + 15) // 16):
                    pm = pmm.tile([P, 512], f32, space="PSUM", tag="pm",
                                  name=f"pm2_{h}_{b}")
                    ts = wlist[b * 16:(b + 1) * 16]
                    for s4 in range(0, len(ts), 4):
                        sub = ts[s4:s4 + 4]
                        tr = ptr.tile([HID, 512], bf, space="PSUM", tag="tr")
                        for i, t in enumerate(sub):
                            nc.tensor.transpose(
                                out=tr[:, i * P:(i + 1) * P],
                                in_=z_t[:, t * HID:(t + 1) * HID],
                                identity=ident[:])
                        zc = zcpool.tile([HID, 512], bf, tag="zc",
                                         name=f"zc{h}_{b}_{s4}")
                        nc.vector.tensor_copy(out=zc[:, 0:len(sub) * P],
                                              in_=tr[:, 0:len(sub) * P])
                        for i, t in enumerate(sub):
                            nc.tensor.matmul(
                                out=pm[:, (s4 + i) * HID2:(s4 + i + 1) * HID2],
                                lhsT=zc[:, i * P:(i + 1) * P],
                                rhs=w2_t[:], start=True, stop=True)
                    nts = len(ts)
                    b0 = ts[0]
                    nc.vector.tensor_copy(
                        out=u2_t[:, b0 * HID2:(b0 + nts) * HID2],
                        in_=pm[:, 0:nts * HID2])
                    nc.vector.tensor_tensor(
                        out=t2v[:, b0:b0 + nts, :],
                        in0=pm[:, 0:nts * HID2].rearrange("p (t f) -> p t f",
                                                          f=HID2),
                        in1=dinv_t[:, b0:b0 + nts, None]
                            .to_broadcast([P, nts, HID2]),
                        op=mybir.AluOpType.mult)
                nc.sync.dma_start(
                    out=bassm.AP(tensor=tab2_d[:].tensor,
                                 offset=h * HWPC * TBW,
                                 ap=[[WPC * TBW, P], [TBW, HWPC], [1, HID2]]),
                    in_=t2v[:, h * HWPC:(h + 1) * HWPC, :])

            # prebuild the first L2 S bins: pure DVE work that fills the
            # RS1-B bubble (no deps on layer-1 results)
            KPRE = int(_os.environ.get("KPRE", "2"))
            pre_s = [build_s(bi, 2) for bi in range(KPRE)]

            for h in range(2):
                read_agg(agg1, rs1_d, HID, h)
                post(agg1, pre1, dinv_t, HID, z_t, h)
                dense2_half(h)

            edge_stream(tab2_d, acc2_d, rs2_d, HID2, 16, 2, prebuilt=pre_s)

            # ---- post2 per half + fc ----
            # table2 values dinv*u2 give self term dinv2*u2; u2 is unscaled.
            pre2 = precompute_self(u2_t, dinv2_t, t2_t, HID2, "2")
            agg2 = upool.tile([P, WPC * HID2], bf, tag="agg2")
            h2_t = upool.tile([P, WPC * HID2], bf, tag="h2")
            out_t = upool.tile([P, WPC], f32, tag="out")
            for h in range(2):
                read_agg(agg2, rs2_d, HID2, h)
                post(agg2, pre2, dinv_t, HID2, h2_t, h)
                prod = wpool.tile([P, HWPC, HID2], bf, tag="prod",
                                  name=f"prod_{h}")
                nc.vector.tensor_tensor(
                    out=prod[:],
                    in0=h2_t[:, h * HWPC * HID2:(h + 1) * HWPC * HID2]
                        .rearrange("p (w f) -> p w f", f=HID2),
                    in1=fcw_t[:, None, :].to_broadcast([P, HWPC, HID2]),
                    op=mybir.AluOpType.mult)
                nc.vector.reduce_sum(
                    out=out_t[:, h * HWPC:(h + 1) * HWPC, None], in_=prod[:],
                    axis=mybir.AxisListType.X)
            nc.sync.dma_start(out=y_d[:], in_=out_t[:])

    nc.compile()
    return nc


# ----------------------------------------------------------------------
# entry points
# ----------------------------------------------------------------------
def prepare(inputs):
    inputs = {k: np.asarray(v) for k, v in inputs.items()}
    in_maps, consts = host_prep(**inputs)
    nc = build_bass(consts["T"], consts["win_tile0"], consts["bins"],
                    consts["ntiles"])
    return nc, in_maps, consts


def execute(nc, in_maps):
    from concourse.bass_utils import run_bass_kernel_spmd
    return run_bass_kernel_spmd(nc, in_maps, core_ids=list(range(NCORES)))


def unshard(res, consts):
    y = np.zeros((N_NODES, 1), np.float32)
    fcb = consts["fcb"]
    nos = consts["node_of_slot"]
    for c in range(NCORES):
        v = np.asarray(res.results[c]["y"], np.float32).reshape(-1)  # ell order
        valid = nos[c] >= 0
        y[nos[c][valid], 0] = v[valid] + fcb
    return y


def kernel(**inputs):
    nc, in_maps, consts = prepare(inputs)
    res = execute(nc, in_maps)
    return unshard(res, consts)
